# revision 10
# baseline (speedup 1.0000x reference)
"""GQA attention kernel (B=1, S=2048, D=4096, 32 Q heads / 8 KV heads, RoPE,
causal) for 8 Trainium2 NeuronCores.

Sharding: tensor-parallel over heads. Core c owns Q heads 4c..4c+3 and KV head
c (whole GQA group), computes its context slice and a partial o-projection
(rows 512c..512c+511 of Wo); the host sums the 8 partial outputs.

v3 changes vs v2:
- rowsum fully off the PE: DVE accumulates exp tiles; gpsimd
  partition_all_reduce produces the denominator (no ones-matmuls, no
  partition_broadcast, no [1,512] psum bank)
- o-projection emitted as fine-grained filler (2-4 matmuls at a time)
  inside the attention tile loop so the PE never idles on exp/DVE chains
- pT / vnat / v-transpose in bf16 (2x DVE rate, exact causal widths on
  the ctx matmuls)
- chunk-0 phase A densified: K/V(+Q0/Q1 on late pairs) per arriving pair
- mask/ident loads staged on the vector queue after chunk-0 rope ops so
  they don't compete with the critical startup DMA window
"""
import numpy as np
import ml_dtypes
from contextlib import ExitStack

try:  # reuse compiled executables across processes when possible
    import jax
    jax.config.update("jax_compilation_cache_dir", "/tmp/jax_comp_cache")
    jax.config.update("jax_persistent_cache_min_entry_size_bytes", -1)
    jax.config.update("jax_persistent_cache_min_compile_time_secs", 1.0)
except Exception:
    pass

import concourse.bacc as bacc
import concourse.tile as tile
import concourse.mybir as mybir
import concourse.bass_isa as bass_isa
from concourse.bass_utils import run_bass_kernel_spmd

F32 = mybir.dt.float32
F32R = mybir.dt.float32r
BF16 = mybir.dt.bfloat16

S = 2048            # sequence length
D = 4096            # hidden dim
HD = 128            # head dim
NCORES = 8
QH = 4              # q heads per core
KT = D // 128       # 32 contraction tiles for the projections
NCHUNK = S // 512   # 4 sequence chunks of 512
NJT = S // 128      # 16 seq tiles of 128
INV_SQRT_D = float(1.0 / np.sqrt(np.float32(HD)))
NEG_INF = -3.4e38
ROPE_BASE = 10000.0


def _build_nc():
    nc = bacc.Bacc(None)

    # pre-tiled bf16 inputs: hst rows ((icnk*16 + tp)*128 + p) hold the
    # t-PAIR (2*tp, 2*tp+1) side by side -> one DMA per two tiles
    hst_d = nc.dram_tensor("hst", [NCHUNK * 16 * 128, 1024], BF16, kind="ExternalInput")
    # weights bf16, partition-major [128, KT, m] so group loads are contiguous
    wqb_d = nc.dram_tensor("wqb", [128, KT, QH * HD], BF16, kind="ExternalInput")
    wkb_d = nc.dram_tensor("wkb", [128, KT, HD], BF16, kind="ExternalInput")
    wvb_d = nc.dram_tensor("wvb", [128, KT, HD], BF16, kind="ExternalInput")
    wob_d = nc.dram_tensor("wob", [4 * 128, D], BF16, kind="ExternalInput")
    # cos / sign-folded sin, partition-major per chunk [128, NCHUNK, 512]
    cos_d = nc.dram_tensor("cost", [128, NCHUNK, 512], F32, kind="ExternalInput")
    sinn_d = nc.dram_tensor("sinn", [128, NCHUNK, 512], F32, kind="ExternalInput")
    mask_d = nc.dram_tensor("maskt", [128, 4, 512], BF16, kind="ExternalInput")
    ident_d = nc.dram_tensor("ident", [128, 128], BF16, kind="ExternalInput")
    # output bf16, tiled rows ((st*8 + ec)*128 + p)
    out_d = nc.dram_tensor("out", [NJT * 8 * 128, 512], BF16, kind="ExternalOutput")

    with tile.TileContext(nc) as tc, ExitStack() as ctx:
        wpool = ctx.enter_context(tc.tile_pool(name="wpool", bufs=1))
        cpool = ctx.enter_context(tc.tile_pool(name="cpool", bufs=1))
        big = ctx.enter_context(tc.tile_pool(name="bigacts", bufs=1))
        trig = ctx.enter_context(tc.tile_pool(name="trig", bufs=2))
        hsp = ctx.enter_context(tc.tile_pool(name="hsp", bufs=8))
        hs0 = ctx.enter_context(tc.tile_pool(name="hs0", bufs=8))
        chp = ctx.enter_context(tc.tile_pool(name="chp", bufs=4))
        rotp = ctx.enter_context(tc.tile_pool(name="rotp", bufs=1))
        qrp = ctx.enter_context(tc.tile_pool(name="qrp", bufs=4))
        tmpp = ctx.enter_context(tc.tile_pool(name="tmpp", bufs=1))
        ptp = ctx.enter_context(tc.tile_pool(name="ptp", bufs=6))
        rsp = ctx.enter_context(tc.tile_pool(name="rsp", bufs=2))
        rbp = ctx.enter_context(tc.tile_pool(name="rbp", bufs=2))
        osb = ctx.enter_context(tc.tile_pool(name="osb", bufs=4))
        psum = ctx.enter_context(tc.tile_pool(name="psum", bufs=8, space="PSUM"))

        # ---- resident weights & constants (bf16, loaded directly) ----
        wq_sb = wpool.tile([128, KT, QH * HD], BF16, tag="wq")
        wk_sb = wpool.tile([128, KT, HD], BF16, tag="wk")
        wv_sb = wpool.tile([128, KT, HD], BF16, tag="wv")
        wo_sb = wpool.tile([128, 4, D], BF16, tag="wo")

        mask_sb = cpool.tile([128, 4, 512], BF16, tag="mask")
        ident_sb = cpool.tile([128, 128], BF16, tag="ident")

        # per-chunk tiles: one [128,512] each per chunk so old-chunk reads
        # never depend on the current chunk's writes (false-dep avoidance)
        krope_cs = [big.tile([128, 512], F32R, tag=f"krope{i}",
                             name=f"krope{i}") for i in range(NCHUNK)]
        vnat_cs = [big.tile([128, 512], BF16, tag=f"vnat{i}",
                            name=f"vnat{i}") for i in range(NCHUNK)]
        ctx_cs = [big.tile([128, QH, 512], BF16, tag=f"ctx{i}",
                           name=f"ctx{i}") for i in range(NCHUNK)]

        # --- weight group loads (scalar queue; chunk-0 only) ---
        def wq_load(g):  # 4 groups of 8 t-tiles, 1 MB each
            nc.scalar.dma_start(out=wq_sb[:, 8 * g:8 * g + 8, :],
                                in_=wqb_d[:, 8 * g:8 * g + 8, :])

        nc.scalar.dma_start(out=wk_sb[:, 0:2, :], in_=wkb_d[:, 0:2, :])
        nc.scalar.dma_start(out=wv_sb[:, 0:2, :], in_=wvb_d[:, 0:2, :])
        nc.scalar.dma_start(out=wk_sb[:, 2:16, :], in_=wkb_d[:, 2:16, :])
        nc.scalar.dma_start(out=wv_sb[:, 2:16, :], in_=wvb_d[:, 2:16, :])

        def _late_loads(phase, tp):
            # chunk-0 only: stream remaining weights
            if phase == 0:
                if tp == 1:
                    wq_load(0)
                elif tp == 2:
                    nc.scalar.dma_start(out=wk_sb[:, 16:32, :],
                                        in_=wkb_d[:, 16:32, :])
                    nc.scalar.dma_start(out=wv_sb[:, 16:32, :],
                                        in_=wvb_d[:, 16:32, :])
                elif tp == 3:
                    wq_load(1)
                elif tp == 6:
                    wq_load(2)
            else:
                if tp == 1:
                    wq_load(3)

        hst_prefetched = {}

        # ---- fine-grained o-projection filler ----
        # each unit (st, ecp) = 2 halves x 4 matmuls + psum->sbuf copy; the
        # paired DMA goes out per half. pe_filler(n) emits up to n matmuls,
        # continuing wherever the cursor left off.
        out_r = out_d[:, :].rearrange("(n p) f -> p n f", p=128)
        _ost = {"st": 0, "ecp": 0, "half": 0, "jt": 0, "max_st": -1,
                "q": 0, "ot": None, "oacc": None, "done": False}

        def pe_filler(budget):
            n = 0
            while n < budget and not _ost["done"]:
                st, ecp, half, jt = (_ost["st"], _ost["ecp"], _ost["half"],
                                     _ost["jt"])
                if st > _ost["max_st"]:
                    break
                if half == 0 and jt == 0:
                    _ost["ot"] = osb.tile([128, 2, 512], BF16, tag="ot",
                                          name=f"ot{st}_{ecp}")
                if jt == 0:
                    _ost["oacc"] = psum.tile([128, 512], F32, tag="ps",
                                             name=f"o{st}_{ecp}_{half}")
                ec = ecp * 2 + half
                nc.tensor.matmul(
                    _ost["oacc"][:],
                    ctx_cs[st // 4][:, jt, (st % 4) * 128:(st % 4 + 1) * 128],
                    wo_sb[:, jt, ec * 512:(ec + 1) * 512],
                    start=(jt == 0), stop=(jt == 3))
                n += 1
                if jt < 3:
                    _ost["jt"] = jt + 1
                    continue
                # half complete: copy out + DMA this half
                if half == 1:
                    nc.scalar.copy(_ost["ot"][:, half, :], _ost["oacc"][:])
                else:
                    nc.vector.tensor_copy(_ost["ot"][:, half, :],
                                          _ost["oacc"][:])
                n0 = st * 8 + ecp * 2 + half
                eng = nc.sync if _ost["q"] % 2 == 0 else nc.gpsimd
                _ost["q"] += 1
                eng.dma_start(out=out_r[:, n0:n0 + 1, :],
                              in_=_ost["ot"][:, half:half + 1, :])
                _ost["jt"] = 0
                if half == 0:
                    _ost["half"] = 1
                else:
                    _ost["half"] = 0
                    if ecp < 3:
                        _ost["ecp"] = ecp + 1
                    else:
                        _ost["ecp"] = 0
                        if st + 1 < NJT:
                            _ost["st"] = st + 1
                        else:
                            _ost["done"] = True
            return n

        def hst_pair_load(icnk, tp):
            key = (icnk, tp)
            if key in hst_prefetched:
                return hst_prefetched.pop(key)
            h = hsp.tile([128, 2, 512], BF16, tag="hst", name=f"hst{icnk}_{tp}")
            r0 = (icnk * 16 + tp) * 128
            nc.sync.dma_start(out=h[:], in_=hst_d[r0:r0 + 128, :])
            return h

        # ---- fused per-chunk pipeline ----
        for icnk in range(NCHUNK):
            # per-chunk trig slices (gpsimd queue, contiguous per partition)
            cos_t = trig.tile([128, 512], F32, tag="cos", name=f"cos{icnk}")
            sinn_t = trig.tile([128, 512], F32, tag="sinn", name=f"sinn{icnk}")
            nc.gpsimd.dma_start(out=cos_t[:], in_=cos_d[:, icnk, :])
            nc.gpsimd.dma_start(out=sinn_t[:], in_=sinn_d[:, icnk, :])

            # joint qkv projection for this chunk: 6 accumulators (q0..q3, k, v)
            accs = [psum.tile([128, 512], F32, tag="ps", name=f"acc{icnk}_{i}")
                    for i in range(6)]
            if icnk == 0:
                # phase A: K/V (+Q0/Q1 on late pairs) — needs just wk/wv(+wq g0)
                # and the first half of hst, so the PE starts early and stays
                # denser while the remaining weights stream in
                apairs = []
                for tp in range(8):
                    hA = hs0.tile([128, 2, 512], BF16, tag="hs0",
                                  name=f"hsA{tp}")
                    nc.sync.dma_start(out=hA[:], in_=hst_d[tp * 128:
                                                           (tp + 1) * 128, :])
                    apairs.append(hA)
                    _late_loads(0, tp)
                    for tt in range(2):
                        t = 2 * tp + tt
                        nc.tensor.matmul(accs[4][:], wk_sb[:, t, :],
                                         hA[:, tt, :],
                                         start=(t == 0), stop=False)
                        nc.tensor.matmul(accs[5][:], wv_sb[:, t, :],
                                         hA[:, tt, :],
                                         start=(t == 0), stop=False)
                        if tp >= 4:  # q0/q1 join once wq g0/g1 have landed
                            for m in (0, 1):
                                nc.tensor.matmul(
                                    accs[m][:],
                                    wq_sb[:, t, m * HD:(m + 1) * HD],
                                    hA[:, tt, :],
                                    start=(t == 8), stop=False)
                for tp in range(16):
                    if tp < 8:
                        hst_t = apairs[tp]
                    else:
                        hst_t = hst_pair_load(0, tp)
                    _late_loads(1, tp)
                    for tt in range(2):
                        t = 2 * tp + tt
                        morder = ((4, 0, 5, 1, 2, 3) if t == KT - 1
                                  else (0, 1, 2, 3))
                        for m in morder:
                            if m < 4:
                                if m < 2 and 8 <= t < 16:
                                    continue  # done in phase A
                                # q0/q1 opened their psum group in phase A
                                nc.tensor.matmul(
                                    accs[m][:], wq_sb[:, t, m * HD:(m + 1) * HD],
                                    hst_t[:, tt, :],
                                    start=(t == 0 and m >= 2),
                                    stop=(t == KT - 1))
                            elif t >= 16:
                                lhsT = wk_sb[:, t, :] if m == 4 else wv_sb[:, t, :]
                                nc.tensor.matmul(accs[m][:], lhsT,
                                                 hst_t[:, tt, :],
                                                 start=False, stop=(t == KT - 1))
                        if 16 <= t < KT - 1:
                            for m in (4, 5):
                                lhsT = wk_sb[:, t, :] if m == 4 else wv_sb[:, t, :]
                                nc.tensor.matmul(accs[m][:], lhsT,
                                                 hst_t[:, tt, :],
                                                 start=False, stop=False)
            else:
                for tp in range(16):
                    hst_t = hst_pair_load(icnk, tp)
                    if icnk == 1 and tp in (4, 8):
                        jt0 = 0 if tp == 4 else 2
                        for jt in (jt0, jt0 + 1):
                            nc.gpsimd.dma_start(
                                out=wo_sb[:, jt, :],
                                in_=wob_d[jt * 128:(jt + 1) * 128, :])
                    for tt in range(2):
                        t = 2 * tp + tt
                        morder = ((4, 0, 5, 1, 2, 3) if t == KT - 1
                                  else (0, 1, 2, 3, 4, 5))
                        for m in morder:
                            if m < 4:
                                lhsT = wq_sb[:, t, m * HD:(m + 1) * HD]
                            elif m == 4:
                                lhsT = wk_sb[:, t, :]
                            else:
                                lhsT = wv_sb[:, t, :]
                            nc.tensor.matmul(accs[m][:], lhsT, hst_t[:, tt, :],
                                             start=(t == 0), stop=(t == KT - 1))

            def evac(m, eng="v", dt=F32R):
                ch = chp.tile([128, 512], dt, tag="ch" if dt == F32R else "chb",
                              name=f"ch{icnk}_{m}",
                              bufs=None if dt == F32R else 2)
                if eng == "s":  # scalar engine: parallel to DVE at boundaries
                    nc.scalar.copy(ch[:], accs[m][:])
                else:
                    nc.vector.tensor_copy(ch[:], accs[m][:])
                return ch

            def rope_into(ch, dest_ap, name):
                # rotate_half via partition-shifted copies; sign folded in sinn
                rot = rotp.tile([128, 512], F32, tag="rot", name=f"rot{name}")
                nc.vector.tensor_copy(rot[0:64, :], ch[64:128, :].bitcast(F32))
                nc.vector.tensor_copy(rot[64:128, :], ch[0:64, :].bitcast(F32))
                t1 = tmpp.tile([128, 512], F32, tag="t1", name=f"t1{name}")
                nc.vector.tensor_mul(t1[:], ch[:].bitcast(F32), cos_t[:])
                t2 = tmpp.tile([128, 512], F32, tag="t2", name=f"t2{name}")
                nc.vector.tensor_mul(t2[:], rot[:], sinn_t[:])
                nc.vector.tensor_add(dest_ap, t1[:], t2[:])

            def rope_q(m):
                qr = qrp.tile([128, 512], F32R, tag="qrp", name=f"qr{icnk}_{m}")
                rope_into(chs[m], qr[:], f"q{icnk}_{m}")
                return qr

            # evacuate ALL psum accumulators upfront (frees banks for the
            # attention tiles; lazy evac deadlocks the 8-slot psum rotation),
            # but compose ropes lazily per head so DVE runs just ahead of PE.
            qrs = [None] * QH
            chs = {}
            if icnk == 0:
                chs[4] = evac(4, "s")
                chs[0] = evac(0, "s")
                chs[5] = evac(5, "s", BF16)
                rope_into(chs[4], krope_cs[icnk][:], f"k{icnk}")
                qrs[0] = qrp.tile([128, 512], F32R, tag="qrp",
                                  name=f"qr{icnk}_0")
                rope_into(chs[0], qrs[0][:], f"q{icnk}_0")
                # stage mask/ident loads here on the scalar queue: they run
                # after chunk-0's evacs, clear of the startup DMA crunch
                nc.scalar.dma_start(out=mask_sb[:], in_=mask_d[:, :, :])
                nc.scalar.dma_start(out=ident_sb[:], in_=ident_d[:, :])
            else:
                chs[0] = evac(0, "s")
                chs[4] = evac(4, "s")
                chs[5] = evac(5, "s", BF16)
                qrs[0] = qrp.tile([128, 512], F32R, tag="qrp",
                                  name=f"qr{icnk}_0")
                rope_into(chs[0], qrs[0][:], f"q{icnk}_0")
                rope_into(chs[4], krope_cs[icnk][:], f"k{icnk}")
            ch_v = chs[5]
            vt_ps = psum.tile([128, 512], BF16, tag="ps", name=f"vt{icnk}",
                              padded_shape=[128, 1024])
            for tt in range(4):
                nc.tensor.matmul(vt_ps[:, tt * 128:(tt + 1) * 128],
                                 ch_v[:, tt * 128:(tt + 1) * 128],
                                 ident_sb[:], is_transpose=True,
                                 start=True, stop=True)
            for tt in range(4):
                nc.vector.tensor_copy(vnat_cs[icnk][:, tt * 128:(tt + 1) * 128],
                                      vt_ps[:, tt * 128:(tt + 1) * 128])
            for m in (1, 2, 3):
                chs[m] = evac(m)
            if icnk >= 1:
                pe_filler(6)

            # attention for the 4 heads, query chunk = icnk (keys 0..4icnk+3)
            def attention(h):
                qr = qrs[h]
                ctx_acc = psum.tile([128, 512], F32, tag="ps",
                                    name=f"ctx{icnk}_{h}")
                acc_rs = rsp.tile([128, 512], F32, tag="rs",
                                  name=f"rs{icnk}_{h}")
                jt_max = icnk * 4 + 3
                pending = []
                LOOKAHEAD = 2

                def consume(item, last):
                    jt, lo, pT = item
                    r = jt - icnk * 4
                    loc = max(lo, 128 * r)  # bf16: exact causal width
                    nc.tensor.matmul(
                        ctx_acc[:, loc:512],
                        vnat_cs[jt // 4][:, (jt % 4) * 128:(jt % 4 + 1) * 128],
                        pT[:, loc:512],
                        start=(jt == 0), stop=last,
                        skip_group_check=True)

                for jt in range(jt_max + 1):
                    r = jt - icnk * 4
                    lo = 0 if r <= 0 else (128 if r == 1 else 256)
                    sT = psum.tile([128, 512], F32, tag="ps",
                                   name=f"sT{icnk}_{h}_{jt}")
                    nc.tensor.matmul(
                        sT[:, lo:512],
                        krope_cs[jt // 4][:, (jt % 4) * 128:(jt % 4 + 1) * 128],
                        qr[:, lo:512], start=True, stop=True)
                    if icnk >= 1:
                        pe_filler(2)
                    if len(pending) >= LOOKAHEAD:
                        consume(pending.pop(0), False)
                    if r >= 0:  # diagonal: apply causal mask
                        nc.vector.tensor_add(sT[:, lo:512], sT[:, lo:512],
                                             mask_sb[:, r, lo:512])
                    pT = ptp.tile([128, 512], BF16, tag="pt",
                                  name=f"pt{icnk}_{h}_{jt}")
                    nc.scalar.activation(out=pT[:, lo:512], in_=sT[:, lo:512],
                                         func=mybir.ActivationFunctionType.Exp,
                                         scale=INV_SQRT_D)
                    # rowsum accumulated on DVE (PE stays on matmuls)
                    if jt == 0:
                        nc.vector.tensor_copy(acc_rs[:, :], pT[:, :])
                    else:
                        nc.vector.tensor_add(acc_rs[:, lo:512],
                                             acc_rs[:, lo:512], pT[:, lo:512])
                    pending.append((jt, lo, pT))
                while pending:
                    if icnk >= 1:
                        pe_filler(2)
                    consume(pending.pop(0), len(pending) == 0)

                # denominator: gpsimd all-reduce over partitions, DVE recip
                rball = rbp.tile([128, 512], F32, tag="rball",
                                 name=f"rball{icnk}_{h}")
                nc.gpsimd.partition_all_reduce(rball[:], acc_rs[:], 128,
                                               bass_isa.ReduceOp.add)
                rb = rbp.tile([128, 512], F32, tag="rb", name=f"rb{icnk}_{h}")
                nc.vector.reciprocal_approx_fast(rb[:], rball[:])
                nc.vector.tensor_mul(ctx_cs[icnk][:, h, :], ctx_acc[:], rb[:])

            for h in range(QH):
                if h + 1 < QH:
                    qrs[h + 1] = rope_q(h + 1)  # DVE runs ahead of the PE here
                attention(h)
                if icnk >= 1:
                    pe_filler(10)
                if icnk < NCHUNK - 1 and h == 1:
                    # prefetch next chunk's first hst pairs during attention
                    for tpre in range(6):
                        hh = hsp.tile([128, 2, 512], BF16, tag="hst",
                                      name=f"hst{icnk + 1}_{tpre}")
                        r0 = ((icnk + 1) * 16 + tpre) * 128
                        nc.sync.dma_start(out=hh[:], in_=hst_d[r0:r0 + 128, :])
                        hst_prefetched[(icnk + 1, tpre)] = hh
            # chunk i's ctx complete -> o-proj units for st 4i..4i+3 unlock
            _ost["max_st"] = 4 * icnk + 3

        # ---- drain remaining o-projection units ----
        pe_filler(10 ** 6)

    nc.finalize()
    return nc


_NC_CACHE = None


def _host_tables():
    inv_freq = 1.0 / (ROPE_BASE ** (np.arange(0, HD, 2, dtype=np.float32) / HD))
    pos = np.arange(S, dtype=np.float32)
    freqs = pos[:, None] * inv_freq[None, :].astype(np.float32)   # [S, 64]
    emb = np.concatenate([freqs, freqs], axis=1).astype(np.float32)  # [S, 128]
    cosT = np.ascontiguousarray(np.cos(emb).astype(np.float32).T)  # [128, S]
    sinT = np.ascontiguousarray(np.sin(emb).astype(np.float32).T)
    sinN = sinT.copy()
    sinN[0:64, :] *= -1.0  # sign folded: rot_abs[0:64]=x2, [64:]=x1

    # per-chunk partition-major trig: [128, NCHUNK, 512]
    cosPM = np.ascontiguousarray(cosT.reshape(128, NCHUNK, 512))
    sinnPM = np.ascontiguousarray(sinN.reshape(128, NCHUNK, 512))

    # causal mask for diagonal blocks, transposed orientation [jp, r, if]
    jp = np.arange(128)[:, None, None]
    r = np.arange(4)[None, :, None]
    iF = np.arange(512)[None, None, :]
    mask = np.where(r * 128 + jp <= iF, 0.0, NEG_INF).astype(np.float32)
    mask_bf = mask.astype(ml_dtypes.bfloat16)

    ident = np.eye(128, dtype=np.float32).astype(ml_dtypes.bfloat16)
    return cosPM, sinnPM, mask_bf, ident


def kernel(hidden_states, Wq, Wk, Wv, Wo):
    global _NC_CACHE
    if _NC_CACHE is None:
        _NC_CACHE = _build_nc()
    nc = _NC_CACHE

    hs = np.asarray(hidden_states, dtype=np.float32)
    B = hs.shape[0]
    assert hs.shape == (B, S, D)
    hst = np.ascontiguousarray(hs[0].T).astype(ml_dtypes.bfloat16)  # [D, S]
    # pair-tiled: row (icnk*16 + tp)*128 + p = [t=2tp | t=2tp+1] halves
    hst_t = np.ascontiguousarray(
        hst.reshape(16, 2, 128, NCHUNK, 512).transpose(3, 0, 2, 1, 4)
    ).reshape(NCHUNK * 16 * 128, 1024)
    cosPM, sinnPM, mask_bf, ident = _host_tables()

    Wq = np.asarray(Wq, dtype=np.float32)
    Wk = np.asarray(Wk, dtype=np.float32)
    Wv = np.asarray(Wv, dtype=np.float32)
    Wo = np.asarray(Wo, dtype=np.float32)

    def pm_bf16(w):  # [D, m] -> partition-major [128, KT, m] bf16
        m = w.shape[1]
        return np.ascontiguousarray(
            w.reshape(KT, 128, m).transpose(1, 0, 2)).astype(ml_dtypes.bfloat16)

    in_maps = []
    for c in range(NCORES):
        in_maps.append({
            "hst": hst_t,
            "wqb": pm_bf16(Wq[:, c * QH * HD:(c + 1) * QH * HD]),
            "wkb": pm_bf16(Wk[:, c * HD:(c + 1) * HD]),
            "wvb": pm_bf16(Wv[:, c * HD:(c + 1) * HD]),
            "wob": np.ascontiguousarray(
                Wo[c * QH * HD:(c + 1) * QH * HD, :]).astype(ml_dtypes.bfloat16),
            "cost": cosPM,
            "sinn": sinnPM,
            "maskt": mask_bf,
            "ident": ident,
        })

    import os
    trace = os.environ.get("KERNEL_TRACE") == "1"
    if trace:
        try:
            import antenv.axon_hooks  # noqa: F401  (profiling hook, optional)
        except ImportError:
            trace = False
    res = run_bass_kernel_spmd(nc, in_maps, list(range(NCORES)), trace=trace)
    if trace:
        kernel.last_results = res

    acc = np.zeros((NJT, 8, 128, 512), dtype=np.float64)
    for c in range(NCORES):
        acc += res.results[c]["out"].astype(np.float64).reshape(NJT, 8, 128, 512)
    # [st, ec, p, f] -> [st, p, ec, f] -> [S, D]
    out = acc.transpose(0, 2, 1, 3).reshape(S, D).astype(np.float32)
    return out.reshape(B, S, D)


# revision 23
# speedup vs baseline: 1.0103x; 1.0103x over previous
"""GQA attention kernel (B=1, S=2048, D=4096, 32 Q heads / 8 KV heads, RoPE,
causal) for 8 Trainium2 NeuronCores.

Sharding: tensor-parallel over heads. Core c owns Q heads 4c..4c+3 and KV head
c (whole GQA group), computes its context slice and a partial o-projection
(rows 512c..512c+511 of Wo); the host sums the 8 partial outputs.

v3 changes vs v2:
- rowsum fully off the PE: DVE accumulates exp tiles; gpsimd
  partition_all_reduce produces the denominator (no ones-matmuls, no
  partition_broadcast, no [1,512] psum bank)
- o-projection emitted as fine-grained filler (2-4 matmuls at a time)
  inside the attention tile loop so the PE never idles on exp/DVE chains
- pT / vnat / v-transpose in bf16 (2x DVE rate, exact causal widths on
  the ctx matmuls)
- chunk-0 phase A densified: K/V(+Q0/Q1 on late pairs) per arriving pair
- mask/ident loads staged on the vector queue after chunk-0 rope ops so
  they don't compete with the critical startup DMA window
"""
import numpy as np
import ml_dtypes
from contextlib import ExitStack

try:  # reuse compiled executables across processes when possible
    import jax
    jax.config.update("jax_compilation_cache_dir", "/tmp/jax_comp_cache")
    jax.config.update("jax_persistent_cache_min_entry_size_bytes", -1)
    jax.config.update("jax_persistent_cache_min_compile_time_secs", 1.0)
except Exception:
    pass

import concourse.bacc as bacc
import concourse.tile as tile
import concourse.mybir as mybir
import concourse.bass_isa as bass_isa
from concourse.bass_utils import run_bass_kernel_spmd

F32 = mybir.dt.float32
F32R = mybir.dt.float32r
BF16 = mybir.dt.bfloat16

S = 2048            # sequence length
D = 4096            # hidden dim
HD = 128            # head dim
NCORES = 8
QH = 4              # q heads per core
KT = D // 128       # 32 contraction tiles for the projections
NCHUNK = S // 512   # 4 sequence chunks of 512
NJT = S // 128      # 16 seq tiles of 128
INV_SQRT_D = float(1.0 / np.sqrt(np.float32(HD)))
NEG_INF = -3.4e38
ROPE_BASE = 10000.0


def _build_nc():
    nc = bacc.Bacc(None)

    # pre-tiled bf16 inputs: hst rows ((icnk*16 + tp)*128 + p) hold the
    # t-PAIR (2*tp, 2*tp+1) side by side -> one DMA per two tiles
    hst_d = nc.dram_tensor("hst", [NCHUNK * 16 * 128, 1024], BF16, kind="ExternalInput")
    # weights bf16, partition-major [128, KT, m] so group loads are contiguous
    wqb_d = nc.dram_tensor("wqb", [128, KT, QH * HD], BF16, kind="ExternalInput")
    wkb_d = nc.dram_tensor("wkb", [128, KT, HD], BF16, kind="ExternalInput")
    wvb_d = nc.dram_tensor("wvb", [128, KT, HD], BF16, kind="ExternalInput")
    wob_d = nc.dram_tensor("wob", [4 * 128, D], BF16, kind="ExternalInput")
    # cos / sign-folded sin, partition-major per chunk [128, NCHUNK, 512]
    cos_d = nc.dram_tensor("cost", [128, NCHUNK, 512], F32, kind="ExternalInput")
    sinn_d = nc.dram_tensor("sinn", [128, NCHUNK, 512], F32, kind="ExternalInput")
    mask_d = nc.dram_tensor("maskt", [128, 4, 512], BF16, kind="ExternalInput")
    ident_d = nc.dram_tensor("ident", [128, 128], BF16, kind="ExternalInput")
    ones_d = nc.dram_tensor("ones", [128, 128], F32R, kind="ExternalInput")
    # output bf16, tiled rows ((st*8 + ec)*128 + p)
    out_d = nc.dram_tensor("out", [NJT * 8 * 128, 512], BF16, kind="ExternalOutput")

    with tile.TileContext(nc) as tc, ExitStack() as ctx:
        wpool = ctx.enter_context(tc.tile_pool(name="wpool", bufs=1))
        cpool = ctx.enter_context(tc.tile_pool(name="cpool", bufs=1))
        big = ctx.enter_context(tc.tile_pool(name="bigacts", bufs=1))
        trig = ctx.enter_context(tc.tile_pool(name="trig", bufs=2))
        hsp = ctx.enter_context(tc.tile_pool(name="hsp", bufs=8))
        hs0 = ctx.enter_context(tc.tile_pool(name="hs0", bufs=8))
        chp = ctx.enter_context(tc.tile_pool(name="chp", bufs=4))
        rotp = ctx.enter_context(tc.tile_pool(name="rotp", bufs=1))
        qrp = ctx.enter_context(tc.tile_pool(name="qrp", bufs=4))
        tmpp = ctx.enter_context(tc.tile_pool(name="tmpp", bufs=1))
        ptp = ctx.enter_context(tc.tile_pool(name="ptp", bufs=6))
        rsp = ctx.enter_context(tc.tile_pool(name="rsp", bufs=2))
        smal = ctx.enter_context(tc.tile_pool(name="smal", bufs=2))
        rbp = ctx.enter_context(tc.tile_pool(name="rbp", bufs=2))
        osb = ctx.enter_context(tc.tile_pool(name="osb", bufs=4))
        psum = ctx.enter_context(tc.tile_pool(name="psum", bufs=8, space="PSUM"))

        # ---- resident weights & constants (bf16, loaded directly) ----
        wq_sb = wpool.tile([128, KT, QH * HD], BF16, tag="wq")
        wk_sb = wpool.tile([128, KT, HD], BF16, tag="wk")
        wv_sb = wpool.tile([128, KT, HD], BF16, tag="wv")
        wo_sb = wpool.tile([128, 4, D], BF16, tag="wo")

        mask_sb = cpool.tile([128, 4, 512], BF16, tag="mask")
        ident_sb = cpool.tile([128, 128], BF16, tag="ident")
        ones_sb = cpool.tile([128, 128], F32R, tag="ones")

        # per-chunk tiles: one [128,512] each per chunk so old-chunk reads
        # never depend on the current chunk's writes (false-dep avoidance)
        krope_cs = [big.tile([128, 512], F32R, tag=f"krope{i}",
                             name=f"krope{i}") for i in range(NCHUNK)]
        vnat_cs = [big.tile([128, 512], BF16, tag=f"vnat{i}",
                            name=f"vnat{i}") for i in range(NCHUNK)]
        ctx_cs = [big.tile([128, QH, 512], BF16, tag=f"ctx{i}",
                           name=f"ctx{i}") for i in range(NCHUNK)]

        # --- weight group loads (scalar queue; chunk-0 only) ---
        def wq_load(g):  # 4 groups of 8 t-tiles, 1 MB each
            nc.scalar.dma_start(out=wq_sb[:, 8 * g:8 * g + 8, :],
                                in_=wqb_d[:, 8 * g:8 * g + 8, :])

        nc.scalar.dma_start(out=wk_sb[:, 0:2, :], in_=wkb_d[:, 0:2, :])
        nc.scalar.dma_start(out=wv_sb[:, 0:2, :], in_=wvb_d[:, 0:2, :])
        nc.scalar.dma_start(out=wk_sb[:, 2:16, :], in_=wkb_d[:, 2:16, :])
        nc.scalar.dma_start(out=wv_sb[:, 2:16, :], in_=wvb_d[:, 2:16, :])

        def _late_loads(phase, tp):
            # chunk-0 only: stream remaining weights
            if phase == 0:
                if tp == 1:
                    wq_load(0)
                elif tp == 2:
                    nc.scalar.dma_start(out=wk_sb[:, 16:32, :],
                                        in_=wkb_d[:, 16:32, :])
                    nc.scalar.dma_start(out=wv_sb[:, 16:32, :],
                                        in_=wvb_d[:, 16:32, :])
                elif tp == 3:
                    wq_load(1)
                elif tp == 6:
                    wq_load(2)
            else:
                if tp == 1:
                    wq_load(3)

        hst_prefetched = {}

        # ---- fine-grained o-projection filler ----
        # each unit (st, ecp) = 2 halves x 4 matmuls + psum->sbuf copy; the
        # paired DMA goes out per half. pe_filler(n) emits up to n matmuls,
        # continuing wherever the cursor left off.
        out_r = out_d[:, :].rearrange("(n p) f -> p n f", p=128)
        _ost = {"st": 0, "ecp": 0, "half": 0, "jt": 0, "max_st": -1,
                "q": 0, "ot": None, "oacc": None, "done": False}

        def pe_filler(budget):
            n = 0
            while n < budget and not _ost["done"]:
                st, ecp, half, jt = (_ost["st"], _ost["ecp"], _ost["half"],
                                     _ost["jt"])
                if st > _ost["max_st"]:
                    break
                if half == 0 and jt == 0:
                    _ost["ot"] = osb.tile([128, 2, 512], BF16, tag="ot",
                                          name=f"ot{st}_{ecp}")
                if jt == 0:
                    _ost["oacc"] = psum.tile([128, 512], F32, tag="ps",
                                             name=f"o{st}_{ecp}_{half}")
                ec = ecp * 2 + half
                nc.tensor.matmul(
                    _ost["oacc"][:],
                    ctx_cs[st // 4][:, jt, (st % 4) * 128:(st % 4 + 1) * 128],
                    wo_sb[:, jt, ec * 512:(ec + 1) * 512],
                    start=(jt == 0), stop=(jt == 3))
                n += 1
                if jt < 3:
                    _ost["jt"] = jt + 1
                    continue
                # half complete: copy out + DMA this half
                if half == 1:
                    nc.scalar.copy(_ost["ot"][:, half, :], _ost["oacc"][:])
                else:
                    nc.vector.tensor_copy(_ost["ot"][:, half, :],
                                          _ost["oacc"][:])
                n0 = st * 8 + ecp * 2 + half
                eng = nc.gpsimd if _ost["q"] % 4 == 3 else nc.sync
                _ost["q"] += 1
                eng.dma_start(out=out_r[:, n0:n0 + 1, :],
                              in_=_ost["ot"][:, half:half + 1, :])
                _ost["jt"] = 0
                if half == 0:
                    _ost["half"] = 1
                else:
                    _ost["half"] = 0
                    if ecp < 3:
                        _ost["ecp"] = ecp + 1
                    else:
                        _ost["ecp"] = 0
                        if st + 1 < NJT:
                            _ost["st"] = st + 1
                        else:
                            _ost["done"] = True
            return n

        def hst_pair_load(icnk, tp):
            key = (icnk, tp)
            if key in hst_prefetched:
                return hst_prefetched.pop(key)
            h = hsp.tile([128, 2, 512], BF16, tag="hst", name=f"hst{icnk}_{tp}")
            r0 = (icnk * 16 + tp) * 128
            nc.sync.dma_start(out=h[:], in_=hst_d[r0:r0 + 128, :])
            return h

        # ---- fused per-chunk pipeline ----
        for icnk in range(NCHUNK):
            # per-chunk trig slices (gpsimd queue, contiguous per partition)
            cos_t = trig.tile([128, 512], F32, tag="cos", name=f"cos{icnk}")
            sinn_t = trig.tile([128, 512], F32, tag="sinn", name=f"sinn{icnk}")
            nc.gpsimd.dma_start(out=cos_t[:], in_=cos_d[:, icnk, :])
            nc.gpsimd.dma_start(out=sinn_t[:], in_=sinn_d[:, icnk, :])

            # joint qkv projection for this chunk: 6 accumulators (q0..q3, k, v)
            accs = [psum.tile([128, 512], F32, tag="ps", name=f"acc{icnk}_{i}")
                    for i in range(6)]
            if icnk == 0:
                # phase A: K/V (+Q0/Q1 on late pairs) — needs just wk/wv(+wq g0)
                # and the first half of hst, so the PE starts early and stays
                # denser while the remaining weights stream in
                apairs = []
                for tp in range(8):
                    hA = hs0.tile([128, 2, 512], BF16, tag="hs0",
                                  name=f"hsA{tp}")
                    nc.sync.dma_start(out=hA[:], in_=hst_d[tp * 128:
                                                           (tp + 1) * 128, :])
                    apairs.append(hA)
                    _late_loads(0, tp)
                    for tt in range(2):
                        t = 2 * tp + tt
                        nc.tensor.matmul(accs[4][:], wk_sb[:, t, :],
                                         hA[:, tt, :],
                                         start=(t == 0), stop=False)
                        nc.tensor.matmul(accs[5][:], wv_sb[:, t, :],
                                         hA[:, tt, :],
                                         start=(t == 0), stop=False)
                        if tp >= 4:  # q0/q1 join, lagged 2 pairs so a late
                            tl = t - 4  # wq arrival can't block K/V
                            for m in (0, 1):
                                nc.tensor.matmul(
                                    accs[m][:],
                                    wq_sb[:, tl, m * HD:(m + 1) * HD],
                                    apairs[tp - 2][:, tt, :],
                                    start=(tl == 4), stop=False)
                for tp in range(16):
                    if tp < 8:
                        hst_t = apairs[tp]
                    else:
                        hst_t = hst_pair_load(0, tp)
                    _late_loads(1, tp)
                    for tt in range(2):
                        t = 2 * tp + tt
                        morder = ((4, 0, 5, 1, 2, 3) if t == KT - 1
                                  else (0, 1, 2, 3))
                        for m in morder:
                            if m < 4:
                                if m < 2 and 4 <= t < 12:
                                    continue  # done in phase A
                                # q0/q1 opened their psum group in phase A
                                nc.tensor.matmul(
                                    accs[m][:], wq_sb[:, t, m * HD:(m + 1) * HD],
                                    hst_t[:, tt, :],
                                    start=(t == 0 and m >= 2),
                                    stop=(t == KT - 1))
                            elif t >= 16:
                                lhsT = wk_sb[:, t, :] if m == 4 else wv_sb[:, t, :]
                                nc.tensor.matmul(accs[m][:], lhsT,
                                                 hst_t[:, tt, :],
                                                 start=False, stop=(t == KT - 1))
                        if 16 <= t < KT - 1:
                            for m in (4, 5):
                                lhsT = wk_sb[:, t, :] if m == 4 else wv_sb[:, t, :]
                                nc.tensor.matmul(accs[m][:], lhsT,
                                                 hst_t[:, tt, :],
                                                 start=False, stop=False)
            else:
                for tp in range(16):
                    hst_t = hst_pair_load(icnk, tp)
                    for tt in range(2):
                        t = 2 * tp + tt
                        morder = ((4, 0, 5, 1, 2, 3) if t == KT - 1
                                  else (0, 1, 2, 3, 4, 5))
                        for m in morder:
                            if m < 4:
                                lhsT = wq_sb[:, t, m * HD:(m + 1) * HD]
                            elif m == 4:
                                lhsT = wk_sb[:, t, :]
                            else:
                                lhsT = wv_sb[:, t, :]
                            nc.tensor.matmul(accs[m][:], lhsT, hst_t[:, tt, :],
                                             start=(t == 0), stop=(t == KT - 1))

            def evac(m, eng="v", dt=F32R):
                ch = chp.tile([128, 512], dt, tag="ch" if dt == F32R else "chb",
                              name=f"ch{icnk}_{m}",
                              bufs=None if dt == F32R else 2)
                if eng == "s":  # scalar engine: parallel to DVE at boundaries
                    nc.scalar.copy(ch[:], accs[m][:])
                else:
                    nc.vector.tensor_copy(ch[:], accs[m][:])
                return ch

            def rope_into(ch, dest_ap, name):
                # rotate_half via partition-shifted copies; sign folded in sinn
                rot = rotp.tile([128, 512], F32, tag="rot", name=f"rot{name}")
                nc.vector.tensor_copy(rot[0:64, :], ch[64:128, :].bitcast(F32))
                nc.vector.tensor_copy(rot[64:128, :], ch[0:64, :].bitcast(F32))
                t1 = tmpp.tile([128, 512], F32, tag="t1", name=f"t1{name}")
                nc.vector.tensor_mul(t1[:], ch[:].bitcast(F32), cos_t[:])
                t2 = tmpp.tile([128, 512], F32, tag="t2", name=f"t2{name}")
                nc.vector.tensor_mul(t2[:], rot[:], sinn_t[:])
                nc.vector.tensor_add(dest_ap, t1[:], t2[:])

            def rope_q(m):
                qr = qrp.tile([128, 512], F32R, tag="qrp", name=f"qr{icnk}_{m}")
                rope_into(chs[m], qr[:], f"q{icnk}_{m}")
                return qr

            # evacuate ALL psum accumulators upfront (frees banks for the
            # attention tiles; lazy evac deadlocks the 8-slot psum rotation),
            # but compose ropes lazily per head so DVE runs just ahead of PE.
            qrs = [None] * QH
            chs = {}
            if icnk == 0:
                chs[4] = evac(4, "s")
                chs[0] = evac(0, "s")
                chs[5] = evac(5, "s", BF16)
                rope_into(chs[4], krope_cs[icnk][:], f"k{icnk}")
                qrs[0] = qrp.tile([128, 512], F32R, tag="qrp",
                                  name=f"qr{icnk}_0")
                rope_into(chs[0], qrs[0][:], f"q{icnk}_0")
                # stage table + wo loads here on the scalar queue: they run
                # after chunk-0's evacs, clear of the startup DMA crunch
                nc.scalar.dma_start(out=ident_sb[:], in_=ident_d[:, :])
                nc.scalar.dma_start(out=ones_sb[:], in_=ones_d[:, :])
                nc.scalar.dma_start(out=mask_sb[:], in_=mask_d[:, :, :])
                for jt in range(4):
                    nc.scalar.dma_start(out=wo_sb[:, jt, :],
                                        in_=wob_d[jt * 128:(jt + 1) * 128, :])
            else:
                chs[0] = evac(0, "s")
                chs[4] = evac(4, "s")
                chs[5] = evac(5, "s", BF16)
                qrs[0] = qrp.tile([128, 512], F32R, tag="qrp",
                                  name=f"qr{icnk}_0")
                rope_into(chs[0], qrs[0][:], f"q{icnk}_0")
                rope_into(chs[4], krope_cs[icnk][:], f"k{icnk}")
            ch_v = chs[5]
            vt_ps = psum.tile([128, 512], BF16, tag="ps", name=f"vt{icnk}",
                              padded_shape=[128, 1024])
            for tt in range(4):
                nc.tensor.matmul(vt_ps[:, tt * 128:(tt + 1) * 128],
                                 ch_v[:, tt * 128:(tt + 1) * 128],
                                 ident_sb[:], is_transpose=True,
                                 start=True, stop=True)
            for tt in range(4):
                nc.vector.tensor_copy(vnat_cs[icnk][:, tt * 128:(tt + 1) * 128],
                                      vt_ps[:, tt * 128:(tt + 1) * 128])
            for m in (1, 2, 3):
                chs[m] = evac(m)
            if icnk >= 1:
                pe_filler(6)

            # attention for the 4 heads, query chunk = icnk (keys 0..4icnk+3)
            def attention(h):
                qr = qrs[h]
                ctx_acc = psum.tile([128, 512], F32, tag="ps",
                                    name=f"ctx{icnk}_{h}")
                acc_rs = rsp.tile([128, 512], F32R, tag="rs",
                                  name=f"rs{icnk}_{h}")
                jt_max = icnk * 4 + 3
                pending = []
                LOOKAHEAD = 2

                def consume(item, last):
                    jt, lo, pT = item
                    r = jt - icnk * 4
                    loc = max(lo, 128 * r)  # bf16: exact causal width
                    nc.tensor.matmul(
                        ctx_acc[:, loc:512],
                        vnat_cs[jt // 4][:, (jt % 4) * 128:(jt % 4 + 1) * 128],
                        pT[:, loc:512],
                        start=(jt == 0), stop=last,
                        skip_group_check=True)

                for jt in range(jt_max + 1):
                    r = jt - icnk * 4
                    lo = 0 if r <= 0 else (128 if r == 1 else 256)
                    sT = psum.tile([128, 512], F32, tag="ps",
                                   name=f"sT{icnk}_{h}_{jt}")
                    nc.tensor.matmul(
                        sT[:, lo:512],
                        krope_cs[jt // 4][:, (jt % 4) * 128:(jt % 4 + 1) * 128],
                        qr[:, lo:512], start=True, stop=True)
                    if icnk >= 1:
                        pe_filler(2)
                    if len(pending) >= LOOKAHEAD:
                        consume(pending.pop(0), False)
                    if r >= 0:  # diagonal: apply causal mask
                        nc.vector.tensor_add(sT[:, lo:512], sT[:, lo:512],
                                             mask_sb[:, r, lo:512])
                    pT = ptp.tile([128, 512], BF16, tag="pt",
                                  name=f"pt{icnk}_{h}_{jt}")
                    nc.scalar.activation(out=pT[:, lo:512], in_=sT[:, lo:512],
                                         func=mybir.ActivationFunctionType.Exp,
                                         scale=INV_SQRT_D)
                    # rowsum accumulated on DVE (PE stays on matmuls)
                    if jt == 0:
                        nc.vector.tensor_copy(acc_rs[:, :], pT[:, :])
                    else:
                        nc.vector.tensor_add(acc_rs[:, lo:512],
                                             acc_rs[:, lo:512], pT[:, lo:512])
                    pending.append((jt, lo, pT))
                while pending:
                    if icnk >= 1:
                        pe_filler(2)
                    consume(pending.pop(0), len(pending) == 0)

                # denominator: one [1,512] ones-matmul on PE, recip, broadcast
                rs_ps = psum.tile([1, 512], F32, tag="ps",
                                  name=f"rsps{icnk}_{h}")
                nc.tensor.matmul(rs_ps[:], ones_sb[:, 0:1], acc_rs[:],
                                 start=True, stop=True,
                                 skip_group_check=True)
                recip = smal.tile([1, 512], F32, tag="recip",
                                  name=f"recip{icnk}_{h}")
                nc.vector.reciprocal_approx_fast(recip[:], rs_ps[:])
                rb = rbp.tile([128, 512], F32, tag="rb", name=f"rb{icnk}_{h}")
                nc.gpsimd.partition_broadcast(rb[:], recip[:])
                nc.vector.tensor_mul(ctx_cs[icnk][:, h, :], ctx_acc[:], rb[:])

            for h in range(QH):
                if h + 1 < QH:
                    qrs[h + 1] = rope_q(h + 1)  # DVE runs ahead of the PE here
                attention(h)
                if icnk >= 1:
                    pe_filler(10)
                if icnk < NCHUNK - 1 and h == 1:
                    # prefetch next chunk's first hst pairs during attention
                    for tpre in range(6):
                        hh = hsp.tile([128, 2, 512], BF16, tag="hst",
                                      name=f"hst{icnk + 1}_{tpre}")
                        r0 = ((icnk + 1) * 16 + tpre) * 128
                        nc.sync.dma_start(out=hh[:], in_=hst_d[r0:r0 + 128, :])
                        hst_prefetched[(icnk + 1, tpre)] = hh
            # chunk i's ctx complete -> o-proj units for st 4i..4i+3 unlock
            _ost["max_st"] = 4 * icnk + 3

        # ---- drain remaining o-projection units ----
        pe_filler(10 ** 6)

    nc.finalize()
    return nc


_NC_CACHE = None


def _host_tables():
    inv_freq = 1.0 / (ROPE_BASE ** (np.arange(0, HD, 2, dtype=np.float32) / HD))
    pos = np.arange(S, dtype=np.float32)
    freqs = pos[:, None] * inv_freq[None, :].astype(np.float32)   # [S, 64]
    emb = np.concatenate([freqs, freqs], axis=1).astype(np.float32)  # [S, 128]
    cosT = np.ascontiguousarray(np.cos(emb).astype(np.float32).T)  # [128, S]
    sinT = np.ascontiguousarray(np.sin(emb).astype(np.float32).T)
    sinN = sinT.copy()
    sinN[0:64, :] *= -1.0  # sign folded: rot_abs[0:64]=x2, [64:]=x1

    # per-chunk partition-major trig: [128, NCHUNK, 512]
    cosPM = np.ascontiguousarray(cosT.reshape(128, NCHUNK, 512))
    sinnPM = np.ascontiguousarray(sinN.reshape(128, NCHUNK, 512))

    # causal mask for diagonal blocks, transposed orientation [jp, r, if]
    jp = np.arange(128)[:, None, None]
    r = np.arange(4)[None, :, None]
    iF = np.arange(512)[None, None, :]
    mask = np.where(r * 128 + jp <= iF, 0.0, NEG_INF).astype(np.float32)
    mask_bf = mask.astype(ml_dtypes.bfloat16)

    ident = np.eye(128, dtype=np.float32).astype(ml_dtypes.bfloat16)
    ones = np.ones((128, 128), dtype=np.float32)
    return cosPM, sinnPM, mask_bf, ident, ones


def kernel(hidden_states, Wq, Wk, Wv, Wo):
    global _NC_CACHE
    if _NC_CACHE is None:
        _NC_CACHE = _build_nc()
    nc = _NC_CACHE

    hs = np.asarray(hidden_states, dtype=np.float32)
    B = hs.shape[0]
    assert hs.shape == (B, S, D)
    hst = np.ascontiguousarray(hs[0].T).astype(ml_dtypes.bfloat16)  # [D, S]
    # pair-tiled: row (icnk*16 + tp)*128 + p = [t=2tp | t=2tp+1] halves
    hst_t = np.ascontiguousarray(
        hst.reshape(16, 2, 128, NCHUNK, 512).transpose(3, 0, 2, 1, 4)
    ).reshape(NCHUNK * 16 * 128, 1024)
    cosPM, sinnPM, mask_bf, ident, ones = _host_tables()

    Wq = np.asarray(Wq, dtype=np.float32)
    Wk = np.asarray(Wk, dtype=np.float32)
    Wv = np.asarray(Wv, dtype=np.float32)
    Wo = np.asarray(Wo, dtype=np.float32)

    def pm_bf16(w):  # [D, m] -> partition-major [128, KT, m] bf16
        m = w.shape[1]
        return np.ascontiguousarray(
            w.reshape(KT, 128, m).transpose(1, 0, 2)).astype(ml_dtypes.bfloat16)

    in_maps = []
    for c in range(NCORES):
        in_maps.append({
            "hst": hst_t,
            "wqb": pm_bf16(Wq[:, c * QH * HD:(c + 1) * QH * HD]),
            "wkb": pm_bf16(Wk[:, c * HD:(c + 1) * HD]),
            "wvb": pm_bf16(Wv[:, c * HD:(c + 1) * HD]),
            "wob": np.ascontiguousarray(
                Wo[c * QH * HD:(c + 1) * QH * HD, :]).astype(ml_dtypes.bfloat16),
            "cost": cosPM,
            "sinn": sinnPM,
            "maskt": mask_bf,
            "ident": ident,
            "ones": ones,
        })

    import os
    trace = os.environ.get("KERNEL_TRACE") == "1"
    if trace:
        try:
            import antenv.axon_hooks  # noqa: F401  (profiling hook, optional)
        except ImportError:
            trace = False
    res = run_bass_kernel_spmd(nc, in_maps, list(range(NCORES)), trace=trace)
    if trace:
        kernel.last_results = res

    acc = np.zeros((NJT, 8, 128, 512), dtype=np.float64)
    for c in range(NCORES):
        acc += res.results[c]["out"].astype(np.float64).reshape(NJT, 8, 128, 512)
    # [st, ec, p, f] -> [st, p, ec, f] -> [S, D]
    out = acc.transpose(0, 2, 1, 3).reshape(S, D).astype(np.float32)
    return out.reshape(B, S, D)


# revision 27
# speedup vs baseline: 1.1218x; 1.1104x over previous
"""GQA attention kernel (B=1, S=2048, D=4096, 32 Q heads / 8 KV heads, RoPE,
causal) for 8 Trainium2 NeuronCores.

Sharding: tensor-parallel over heads. Core c owns Q heads 4c..4c+3 and KV head
c (whole GQA group), computes its context slice and a partial o-projection
(rows 512c..512c+511 of Wo); the host sums the 8 partial outputs.

v4 changes vs v2:
- chunk-1's K/V projection matmuls emitted as PE filler inside chunk-0's
  attention (the previously-idle 35us DVE-bound window); chunk-1's own
  loop then only runs Q0-3 + K/V t16-31
- o-projection filler emitted 2-4 matmuls at a time inside the attention
  tile loop (cursor-based; per-unit paired DMA, 3:1 sync:gpsimd triggers)
- pT / vnat / v-transpose in bf16 (2x DVE rate on ctx normalize, exact
  causal widths on ctx matmuls)
- chunk-0 phase A densified with lag-2 q0/q1 matmuls
- wo/mask/ident/ones loads staged on the scalar queue after chunk-0 evacs
  (clear of the startup DMA crunch, ahead of first o-proj use)
"""
import numpy as np
import ml_dtypes
from contextlib import ExitStack

try:  # reuse compiled executables across processes when possible
    import jax
    jax.config.update("jax_compilation_cache_dir", "/tmp/jax_comp_cache")
    jax.config.update("jax_persistent_cache_min_entry_size_bytes", -1)
    jax.config.update("jax_persistent_cache_min_compile_time_secs", 1.0)
except Exception:
    pass

import concourse.bacc as bacc
import concourse.tile as tile
import concourse.mybir as mybir
from concourse.bass_utils import run_bass_kernel_spmd

F32 = mybir.dt.float32
F32R = mybir.dt.float32r
BF16 = mybir.dt.bfloat16

S = 2048            # sequence length
D = 4096            # hidden dim
HD = 128            # head dim
NCORES = 8
QH = 4              # q heads per core
KT = D // 128       # 32 contraction tiles for the projections
NCHUNK = S // 512   # 4 sequence chunks of 512
NJT = S // 128      # 16 seq tiles of 128
INV_SQRT_D = float(1.0 / np.sqrt(np.float32(HD)))
NEG_INF = -3.4e38
ROPE_BASE = 10000.0


def _build_nc():
    nc = bacc.Bacc(None)

    # pre-tiled bf16 inputs: hst rows ((icnk*16 + tp)*128 + p) hold the
    # t-PAIR (2*tp, 2*tp+1) side by side -> one DMA per two tiles
    hst_d = nc.dram_tensor("hst", [NCHUNK * 16 * 128, 1024], BF16, kind="ExternalInput")
    # weights bf16, partition-major [128, KT, m] so group loads are contiguous
    wqb_d = nc.dram_tensor("wqb", [128, KT, QH * HD], BF16, kind="ExternalInput")
    wkb_d = nc.dram_tensor("wkb", [128, KT, HD], BF16, kind="ExternalInput")
    wvb_d = nc.dram_tensor("wvb", [128, KT, HD], BF16, kind="ExternalInput")
    wob_d = nc.dram_tensor("wob", [4 * 128, D], BF16, kind="ExternalInput")
    # cos / sign-folded sin, partition-major per chunk [128, NCHUNK, 512]
    cos_d = nc.dram_tensor("cost", [128, NCHUNK, 512], F32, kind="ExternalInput")
    sinn_d = nc.dram_tensor("sinn", [128, NCHUNK, 512], F32, kind="ExternalInput")
    mask_d = nc.dram_tensor("maskt", [128, 4, 512], BF16, kind="ExternalInput")
    ident_d = nc.dram_tensor("ident", [128, 128], BF16, kind="ExternalInput")
    ones_d = nc.dram_tensor("ones", [128, 128], F32R, kind="ExternalInput")
    onesb_d = nc.dram_tensor("onesb", [128, 128], BF16, kind="ExternalInput")
    # output bf16, tiled rows ((st*8 + ec)*128 + p)
    out_d = nc.dram_tensor("out", [NJT * 8 * 128, 512], BF16, kind="ExternalOutput")

    with tile.TileContext(nc) as tc, ExitStack() as ctx:
        wpool = ctx.enter_context(tc.tile_pool(name="wpool", bufs=1))
        cpool = ctx.enter_context(tc.tile_pool(name="cpool", bufs=1))
        big = ctx.enter_context(tc.tile_pool(name="bigacts", bufs=1))
        trig = ctx.enter_context(tc.tile_pool(name="trig", bufs=2))
        hsp = ctx.enter_context(tc.tile_pool(name="hsp", bufs=10))
        hs0 = ctx.enter_context(tc.tile_pool(name="hs0", bufs=8))
        chp = ctx.enter_context(tc.tile_pool(name="chp", bufs=4))
        rotp = ctx.enter_context(tc.tile_pool(name="rotp", bufs=1))
        qrp = ctx.enter_context(tc.tile_pool(name="qrp", bufs=4))
        tmpp = ctx.enter_context(tc.tile_pool(name="tmpp", bufs=1))
        ptp = ctx.enter_context(tc.tile_pool(name="ptp", bufs=6))
        rsp = ctx.enter_context(tc.tile_pool(name="rsp", bufs=2))
        smal = ctx.enter_context(tc.tile_pool(name="smal", bufs=1))
        rbp = ctx.enter_context(tc.tile_pool(name="rbp", bufs=2))
        osb = ctx.enter_context(tc.tile_pool(name="osb", bufs=4))
        psum = ctx.enter_context(tc.tile_pool(name="psum", bufs=8, space="PSUM"))

        # ---- resident weights & constants (bf16, loaded directly) ----
        wq_sb = wpool.tile([128, KT, QH * HD], BF16, tag="wq")
        wk_sb = wpool.tile([128, KT, HD], BF16, tag="wk")
        wv_sb = wpool.tile([128, KT, HD], BF16, tag="wv")
        wo_sb = wpool.tile([128, 4, D], BF16, tag="wo")

        mask_sb = cpool.tile([128, 4, 512], BF16, tag="mask")
        ident_sb = cpool.tile([128, 128], BF16, tag="ident")
        ones_sb = cpool.tile([128, 128], F32R, tag="ones")
        onesb_sb = cpool.tile([128, 128], BF16, tag="onesb")

        # per-chunk tiles: one [128,512] each per chunk so old-chunk reads
        # never depend on the current chunk's writes (false-dep avoidance)
        krope_cs = [big.tile([128, 512], F32R, tag=f"krope{i}",
                             name=f"krope{i}") for i in range(NCHUNK)]
        vnat_cs = [big.tile([128, 512], BF16, tag=f"vnat{i}",
                            name=f"vnat{i}") for i in range(NCHUNK)]
        ctx_cs = [big.tile([128, QH, 512], BF16, tag=f"ctx{i}",
                           name=f"ctx{i}") for i in range(NCHUNK)]

        # --- weight group loads (scalar queue; chunk-0 only) ---
        def wq_load(g):  # 4 groups of 8 t-tiles, 1 MB each
            nc.scalar.dma_start(out=wq_sb[:, 8 * g:8 * g + 8, :],
                                in_=wqb_d[:, 8 * g:8 * g + 8, :])

        nc.scalar.dma_start(out=wk_sb[:, 0:2, :], in_=wkb_d[:, 0:2, :])
        nc.scalar.dma_start(out=wv_sb[:, 0:2, :], in_=wvb_d[:, 0:2, :])
        nc.scalar.dma_start(out=wk_sb[:, 2:16, :], in_=wkb_d[:, 2:16, :])
        nc.scalar.dma_start(out=wv_sb[:, 2:16, :], in_=wvb_d[:, 2:16, :])

        def _late_loads(phase, tp):
            # chunk-0 only: stream remaining weights
            if phase == 0:
                if tp == 1:
                    wq_load(0)
                elif tp == 2:
                    nc.scalar.dma_start(out=wk_sb[:, 16:32, :],
                                        in_=wkb_d[:, 16:32, :])
                    nc.scalar.dma_start(out=wv_sb[:, 16:32, :],
                                        in_=wvb_d[:, 16:32, :])
                elif tp == 3:
                    wq_load(1)
            else:
                if tp == 0:
                    wq_load(2)
                elif tp == 2:
                    wq_load(3)

        hst_prefetched = {}

        # ---- fine-grained o-projection filler ----
        # each unit (st, ecp) = 2 halves x 4 matmuls; half0 copy on DVE,
        # half1 on scalar, then one paired DMA (3:1 sync:gpsimd triggers).
        # pe_filler(n) emits up to n matmuls, resuming at its cursor.
        out_r = out_d[:, :].rearrange("(n p) f -> p n f", p=128)
        _ost = {"st": 0, "ecp": 0, "half": 0, "jt": 0, "max_st": -1,
                "q": 0, "ot": None, "oacc": None, "done": False}

        def pe_filler(budget):
            n = 0
            while n < budget and not _ost["done"]:
                st, ecp, half, jt = (_ost["st"], _ost["ecp"], _ost["half"],
                                     _ost["jt"])
                if st > _ost["max_st"]:
                    break
                if half == 0 and jt == 0:
                    _ost["ot"] = osb.tile([128, 2, 512], BF16, tag="ot",
                                          name=f"ot{st}_{ecp}")
                if jt == 0:
                    _ost["oacc"] = psum.tile([128, 512], F32, tag="ps",
                                             name=f"o{st}_{ecp}_{half}")
                ec = ecp * 2 + half
                nc.tensor.matmul(
                    _ost["oacc"][:],
                    ctx_cs[st // 4][:, jt, (st % 4) * 128:(st % 4 + 1) * 128],
                    wo_sb[:, jt, ec * 512:(ec + 1) * 512],
                    start=(jt == 0), stop=(jt == 3))
                n += 1
                if jt < 3:
                    _ost["jt"] = jt + 1
                    continue
                if half == 0:
                    nc.vector.tensor_copy(_ost["ot"][:, 0, :], _ost["oacc"][:])
                else:
                    nc.scalar.copy(_ost["ot"][:, 1, :], _ost["oacc"][:])
                    n0 = st * 8 + ecp * 2
                    eng = nc.gpsimd if _ost["q"] % 4 == 3 else nc.sync
                    _ost["q"] += 1
                    eng.dma_start(out=out_r[:, n0:n0 + 2, :], in_=_ost["ot"][:])
                _ost["jt"] = 0
                if half == 0:
                    _ost["half"] = 1
                else:
                    _ost["half"] = 0
                    if ecp < 3:
                        _ost["ecp"] = ecp + 1
                    else:
                        _ost["ecp"] = 0
                        if st + 1 < NJT:
                            _ost["st"] = st + 1
                        else:
                            _ost["done"] = True
            return n

        def hst_pair_load(icnk, tp):
            key = (icnk, tp)
            if key in hst_prefetched:
                return hst_prefetched.pop(key)
            h = hsp.tile([128, 2, 512], BF16, tag="hst", name=f"hst{icnk}_{tp}")
            r0 = (icnk * 16 + tp) * 128
            nc.sync.dma_start(out=h[:], in_=hst_d[r0:r0 + 128, :])
            return h

        # chunk-1 K/V early-start state (filled during chunk-0 attention)
        kv1 = {"accs": None, "pairs": {}, "t": 0}

        def kv1_filler(budget):
            # emit chunk-1 K/V matmuls for prefetched pairs (t = 0..15)
            n = 0
            while n < budget and kv1["t"] < 16:
                t = kv1["t"]
                tp = t // 2
                if tp not in kv1["pairs"]:
                    break
                pair = kv1["pairs"][tp]
                for mi, acc in enumerate(kv1["accs"]):
                    lhsT = wk_sb[:, t, :] if mi == 0 else wv_sb[:, t, :]
                    nc.tensor.matmul(acc[:], lhsT, pair[:, t % 2, :],
                                     start=(t == 0), stop=False)
                    n += 1
                kv1["t"] = t + 1
            return n

        # ---- fused per-chunk pipeline ----
        for icnk in range(NCHUNK):
            # per-chunk trig slices (gpsimd queue, contiguous per partition)
            cos_t = trig.tile([128, 512], F32, tag="cos", name=f"cos{icnk}")
            sinn_t = trig.tile([128, 512], F32, tag="sinn", name=f"sinn{icnk}")
            nc.gpsimd.dma_start(out=cos_t[:], in_=cos_d[:, icnk, :])
            nc.gpsimd.dma_start(out=sinn_t[:], in_=sinn_d[:, icnk, :])

            # joint qkv projection for this chunk: 6 accumulators (q0..q3, k, v)
            if icnk == 1 and kv1["accs"] is not None:
                accs = [psum.tile([128, 512], F32, tag="ps",
                                  name=f"acc{icnk}_{i}") for i in range(4)]
                accs = accs + kv1["accs"]  # K/V groups already open (t 0..15)
            else:
                accs = [psum.tile([128, 512], F32, tag="ps",
                                  name=f"acc{icnk}_{i}") for i in range(6)]
            if icnk == 0:
                # phase A: K/V per arriving pair + lag-2 q0/q1 — the PE
                # starts early and stays denser while weights stream in
                apairs = []
                for tp in range(8):
                    hA = hs0.tile([128, 2, 512], BF16, tag="hs0",
                                  name=f"hsA{tp}")
                    nc.sync.dma_start(out=hA[:], in_=hst_d[tp * 128:
                                                           (tp + 1) * 128, :])
                    apairs.append(hA)
                    _late_loads(0, tp)
                    for tt in range(2):
                        t = 2 * tp + tt
                        nc.tensor.matmul(accs[4][:], wk_sb[:, t, :],
                                         hA[:, tt, :],
                                         start=(t == 0), stop=False)
                        nc.tensor.matmul(accs[5][:], wv_sb[:, t, :],
                                         hA[:, tt, :],
                                         start=(t == 0), stop=False)
                        if tp >= 4:  # q0/q1 join, lagged 2 pairs so a late
                            tl = t - 4  # wq arrival can't block K/V
                            for m in (0, 1):
                                nc.tensor.matmul(
                                    accs[m][:],
                                    wq_sb[:, tl, m * HD:(m + 1) * HD],
                                    apairs[tp - 2][:, tt, :],
                                    start=(tl == 4), stop=False)
                for tp in range(16):
                    if tp < 8:
                        hst_t = apairs[tp]
                    else:
                        hst_t = hst_pair_load(0, tp)
                    _late_loads(1, tp)
                    for tt in range(2):
                        t = 2 * tp + tt
                        morder = ((4, 0, 5, 1, 2, 3) if t == KT - 1
                                  else (0, 1, 2, 3))
                        for m in morder:
                            if m < 4:
                                if m < 2 and 4 <= t < 12:
                                    continue  # done in phase A
                                # q0/q1 opened their psum group in phase A
                                nc.tensor.matmul(
                                    accs[m][:], wq_sb[:, t, m * HD:(m + 1) * HD],
                                    hst_t[:, tt, :],
                                    start=(t == 0 and m >= 2),
                                    stop=(t == KT - 1))
                            elif t >= 16:
                                lhsT = wk_sb[:, t, :] if m == 4 else wv_sb[:, t, :]
                                nc.tensor.matmul(accs[m][:], lhsT,
                                                 hst_t[:, tt, :],
                                                 start=False, stop=(t == KT - 1))
                        if 16 <= t < KT - 1:
                            for m in (4, 5):
                                lhsT = wk_sb[:, t, :] if m == 4 else wv_sb[:, t, :]
                                nc.tensor.matmul(accs[m][:], lhsT,
                                                 hst_t[:, tt, :],
                                                 start=False, stop=False)
            else:
                kv_early = (icnk == 1 and kv1["t"] >= 16)
                for tp in range(16):
                    hst_t = hst_pair_load(icnk, tp)
                    for tt in range(2):
                        t = 2 * tp + tt
                        morder = ((4, 0, 5, 1, 2, 3) if t == KT - 1
                                  else (0, 1, 2, 3, 4, 5))
                        for m in morder:
                            if m < 4:
                                lhsT = wq_sb[:, t, m * HD:(m + 1) * HD]
                            elif m == 4:
                                lhsT = wk_sb[:, t, :]
                            else:
                                lhsT = wv_sb[:, t, :]
                            if m >= 4 and kv_early and t < 16:
                                continue  # K/V t0-15 done in chunk-0 attn
                            nc.tensor.matmul(accs[m][:], lhsT, hst_t[:, tt, :],
                                             start=(t == 0 and not
                                                    (m >= 4 and kv_early)),
                                             stop=(t == KT - 1))

            def evac(m, eng="v", dt=F32R):
                ch = chp.tile([128, 512], dt, tag="ch" if dt == F32R else "chb",
                              name=f"ch{icnk}_{m}",
                              bufs=None if dt == F32R else 2)
                if eng == "s":  # scalar engine: parallel to DVE at boundaries
                    nc.scalar.copy(ch[:], accs[m][:])
                else:
                    nc.vector.tensor_copy(ch[:], accs[m][:])
                return ch

            def rope_into(ch, dest_ap, name):
                # rotate_half via partition-shifted copies; sign folded in sinn
                rot = rotp.tile([128, 512], F32, tag="rot", name=f"rot{name}")
                nc.vector.tensor_copy(rot[0:64, :], ch[64:128, :].bitcast(F32))
                nc.vector.tensor_copy(rot[64:128, :], ch[0:64, :].bitcast(F32))
                t1 = tmpp.tile([128, 512], F32, tag="t1", name=f"t1{name}")
                nc.vector.tensor_mul(t1[:], ch[:].bitcast(F32), cos_t[:])
                t2 = tmpp.tile([128, 512], F32, tag="t2", name=f"t2{name}")
                nc.vector.tensor_mul(t2[:], rot[:], sinn_t[:])
                nc.vector.tensor_add(dest_ap, t1[:], t2[:])

            def rope_q(m):
                qr = qrp.tile([128, 512], F32R, tag="qrp", name=f"qr{icnk}_{m}")
                rope_into(chs[m], qr[:], f"q{icnk}_{m}")
                return qr

            # evacuate ALL psum accumulators upfront (frees banks for the
            # attention tiles; lazy evac deadlocks the 8-slot psum rotation),
            # but compose ropes lazily per head so DVE runs just ahead of PE.
            qrs = [None] * QH
            chs = {}
            if icnk == 0:
                chs[4] = evac(4, "s")
                chs[0] = evac(0, "s")
                chs[5] = evac(5, "s", BF16)
                rope_into(chs[4], krope_cs[icnk][:], f"k{icnk}")
                qrs[0] = qrp.tile([128, 512], F32R, tag="qrp",
                                  name=f"qr{icnk}_0")
                rope_into(chs[0], qrs[0][:], f"q{icnk}_0")
                # stage table loads on the scalar queue: they run after
                # chunk-0's evacs, clear of the startup DMA crunch
                nc.scalar.dma_start(out=ident_sb[:], in_=ident_d[:, :])
                nc.scalar.dma_start(out=mask_sb[:], in_=mask_d[:, :, :])
                nc.scalar.dma_start(out=ones_sb[:], in_=ones_d[:, :])
                nc.scalar.dma_start(out=onesb_sb[:], in_=onesb_d[:, :])
            else:
                chs[0] = evac(0, "s")
                chs[4] = evac(4, "s")
                chs[5] = evac(5, "s", BF16)
                qrs[0] = qrp.tile([128, 512], F32R, tag="qrp",
                                  name=f"qr{icnk}_0")
                rope_into(chs[0], qrs[0][:], f"q{icnk}_0")
                rope_into(chs[4], krope_cs[icnk][:], f"k{icnk}")
            ch_v = chs[5]
            vt_ps = psum.tile([128, 512], BF16, tag="ps", name=f"vt{icnk}",
                              padded_shape=[128, 1024])
            for tt in range(4):
                nc.tensor.matmul(vt_ps[:, tt * 128:(tt + 1) * 128],
                                 ch_v[:, tt * 128:(tt + 1) * 128],
                                 ident_sb[:], is_transpose=True,
                                 start=True, stop=True)
            for tt in range(4):
                nc.vector.tensor_copy(vnat_cs[icnk][:, tt * 128:(tt + 1) * 128],
                                      vt_ps[:, tt * 128:(tt + 1) * 128])
            for m in (1, 2, 3):
                chs[m] = evac(m)
            if icnk == 0:
                # open chunk-1 K/V psum groups; their matmuls fill chunk-0's
                # attention stalls as prefetched chunk-1 pairs arrive
                kv1["accs"] = [psum.tile([128, 512], F32, tag="ps",
                                         name=f"acc1_kv{i}") for i in range(2)]
                for tpre in range(8):
                    hh = hsp.tile([128, 2, 512], BF16, tag="hst",
                                  name=f"hst1_{tpre}")
                    r0 = (16 + tpre) * 128
                    nc.sync.dma_start(out=hh[:], in_=hst_d[r0:r0 + 128, :])
                    hst_prefetched[(1, tpre)] = hh
                    kv1["pairs"][tpre] = hh
            if icnk >= 1:
                pe_filler(6)

            def filler(n):
                if icnk == 0:
                    return kv1_filler(n)
                return pe_filler(n)

            # attention for the 4 heads, query chunk = icnk (keys 0..4icnk+3)
            def attention(h):
                qr = qrs[h]
                ctx_acc = psum.tile([128, 512], F32, tag="ps",
                                    name=f"ctx{icnk}_{h}")
                acc_rs = rsp.tile([128, 512], F32R, tag="rs",
                                  name=f"rs{icnk}_{h}")
                jt_max = icnk * 4 + 3
                pending = []
                LOOKAHEAD = 2

                def consume(item, last):
                    jt, lo, pT = item
                    r = jt - icnk * 4
                    loc = max(lo, 128 * r)  # bf16: exact causal width
                    nc.tensor.matmul(
                        ctx_acc[:, loc:512],
                        vnat_cs[jt // 4][:, (jt % 4) * 128:(jt % 4 + 1) * 128],
                        pT[:, loc:512],
                        start=(jt == 0), stop=last,
                        skip_group_check=True)

                rs_ps = psum.tile([1, 512], F32, tag="ps",
                                  name=f"rsps{icnk}_{h}")
                n_pe_rs = 0
                for jt in range(jt_max + 1):
                    r = jt - icnk * 4
                    lo = 0 if r <= 0 else (128 if r == 1 else 256)
                    sT = psum.tile([128, 512], F32, tag="ps",
                                   name=f"sT{icnk}_{h}_{jt}")
                    nc.tensor.matmul(
                        sT[:, lo:512],
                        krope_cs[jt // 4][:, (jt % 4) * 128:(jt % 4 + 1) * 128],
                        qr[:, lo:512], start=True, stop=True)
                    filler(2)
                    if len(pending) >= LOOKAHEAD:
                        consume(pending.pop(0), False)
                    if r >= 0:  # diagonal: apply causal mask
                        nc.vector.tensor_add(sT[:, lo:512], sT[:, lo:512],
                                             mask_sb[:, r, lo:512])
                    pT = ptp.tile([128, 512], BF16, tag="pt",
                                  name=f"pt{icnk}_{h}_{jt}")
                    nc.scalar.activation(out=pT[:, lo:512], in_=sT[:, lo:512],
                                         func=mybir.ActivationFunctionType.Exp,
                                         scale=INV_SQRT_D)
                    # hybrid rowsum: odd OLD tiles via PE ones-matmul into
                    # rs_ps (sequential groups, one open at a time); the rest
                    # accumulated on DVE (balances engine load in big chunks)
                    if jt == 0:
                        nc.vector.tensor_copy(acc_rs[:, :], pT[:, :])
                    elif r < 0 and jt % 2 == 1:
                        nc.tensor.matmul(rs_ps[:], onesb_sb[:, 0:1], pT[:, :],
                                         start=(n_pe_rs == 0), stop=False,
                                         skip_group_check=True)
                        n_pe_rs += 1
                    else:
                        nc.vector.tensor_add(acc_rs[:, lo:512],
                                             acc_rs[:, lo:512], pT[:, lo:512])
                    pending.append((jt, lo, pT))
                while pending:
                    filler(2)
                    consume(pending.pop(0), len(pending) == 0)

                # final rowsum: add the DVE-accumulated part into rs_ps
                nc.tensor.matmul(rs_ps[:], ones_sb[:, 0:1], acc_rs[:],
                                 start=(n_pe_rs == 0), stop=True,
                                 skip_group_check=True)
                recip = smal.tile([1, 512], F32, tag="recip")
                nc.vector.reciprocal_approx_fast(recip[:], rs_ps[:])
                rb = rbp.tile([128, 512], F32, tag="rb", name=f"rb{icnk}_{h}")
                nc.gpsimd.partition_broadcast(rb[:], recip[:])
                nc.vector.tensor_mul(ctx_cs[icnk][:, h, :], ctx_acc[:], rb[:])

            for h in range(QH):
                if h + 1 < QH:
                    qrs[h + 1] = rope_q(h + 1)  # DVE runs ahead of the PE here
                attention(h)
                filler(4)
                if icnk == 0 and h == 0:
                    # wo streams during the rest of chunk-0's attention
                    for jt in range(4):
                        nc.scalar.dma_start(
                            out=wo_sb[:, jt, :],
                            in_=wob_d[jt * 128:(jt + 1) * 128, :])
                if icnk >= 1 and icnk < NCHUNK - 1 and h == 1:
                    # prefetch next chunk's first hst pairs during attention
                    for tpre in range(6):
                        hh = hsp.tile([128, 2, 512], BF16, tag="hst",
                                      name=f"hst{icnk + 1}_{tpre}")
                        r0 = ((icnk + 1) * 16 + tpre) * 128
                        nc.sync.dma_start(out=hh[:], in_=hst_d[r0:r0 + 128, :])
                        hst_prefetched[(icnk + 1, tpre)] = hh
            # chunk i's ctx complete -> o-proj units for st 4i..4i+3 unlock
            _ost["max_st"] = 4 * icnk + 3

        # ---- drain remaining o-projection units ----
        pe_filler(10 ** 6)

    nc.finalize()
    return nc


_NC_CACHE = None


def _host_tables():
    inv_freq = 1.0 / (ROPE_BASE ** (np.arange(0, HD, 2, dtype=np.float32) / HD))
    pos = np.arange(S, dtype=np.float32)
    freqs = pos[:, None] * inv_freq[None, :].astype(np.float32)   # [S, 64]
    emb = np.concatenate([freqs, freqs], axis=1).astype(np.float32)  # [S, 128]
    cosT = np.ascontiguousarray(np.cos(emb).astype(np.float32).T)  # [128, S]
    sinT = np.ascontiguousarray(np.sin(emb).astype(np.float32).T)
    sinN = sinT.copy()
    sinN[0:64, :] *= -1.0  # sign folded: rot_abs[0:64]=x2, [64:]=x1

    # per-chunk partition-major trig: [128, NCHUNK, 512]
    cosPM = np.ascontiguousarray(cosT.reshape(128, NCHUNK, 512))
    sinnPM = np.ascontiguousarray(sinN.reshape(128, NCHUNK, 512))

    # causal mask for diagonal blocks, transposed orientation [jp, r, if]
    jp = np.arange(128)[:, None, None]
    r = np.arange(4)[None, :, None]
    iF = np.arange(512)[None, None, :]
    mask = np.where(r * 128 + jp <= iF, 0.0, NEG_INF).astype(np.float32)
    mask_bf = mask.astype(ml_dtypes.bfloat16)

    ident = np.eye(128, dtype=np.float32).astype(ml_dtypes.bfloat16)
    ones = np.ones((128, 128), dtype=np.float32)
    onesb = np.ones((128, 128), dtype=np.float32).astype(ml_dtypes.bfloat16)
    return cosPM, sinnPM, mask_bf, ident, ones, onesb


def kernel(hidden_states, Wq, Wk, Wv, Wo):
    global _NC_CACHE
    if _NC_CACHE is None:
        _NC_CACHE = _build_nc()
    nc = _NC_CACHE

    hs = np.asarray(hidden_states, dtype=np.float32)
    B = hs.shape[0]
    assert hs.shape == (B, S, D)
    hst = np.ascontiguousarray(hs[0].T).astype(ml_dtypes.bfloat16)  # [D, S]
    # pair-tiled: row (icnk*16 + tp)*128 + p = [t=2tp | t=2tp+1] halves
    hst_t = np.ascontiguousarray(
        hst.reshape(16, 2, 128, NCHUNK, 512).transpose(3, 0, 2, 1, 4)
    ).reshape(NCHUNK * 16 * 128, 1024)
    cosPM, sinnPM, mask_bf, ident, ones, onesb = _host_tables()

    Wq = np.asarray(Wq, dtype=np.float32)
    Wk = np.asarray(Wk, dtype=np.float32)
    Wv = np.asarray(Wv, dtype=np.float32)
    Wo = np.asarray(Wo, dtype=np.float32)

    def pm_bf16(w):  # [D, m] -> partition-major [128, KT, m] bf16
        m = w.shape[1]
        return np.ascontiguousarray(
            w.reshape(KT, 128, m).transpose(1, 0, 2)).astype(ml_dtypes.bfloat16)

    in_maps = []
    for c in range(NCORES):
        in_maps.append({
            "hst": hst_t,
            "wqb": pm_bf16(Wq[:, c * QH * HD:(c + 1) * QH * HD]),
            "wkb": pm_bf16(Wk[:, c * HD:(c + 1) * HD]),
            "wvb": pm_bf16(Wv[:, c * HD:(c + 1) * HD]),
            "wob": np.ascontiguousarray(
                Wo[c * QH * HD:(c + 1) * QH * HD, :]).astype(ml_dtypes.bfloat16),
            "cost": cosPM,
            "sinn": sinnPM,
            "maskt": mask_bf,
            "ident": ident,
            "ones": ones,
            "onesb": onesb,
        })

    import os
    trace = os.environ.get("KERNEL_TRACE") == "1"
    if trace:
        try:
            import antenv.axon_hooks  # noqa: F401  (profiling hook, optional)
        except ImportError:
            trace = False
    res = run_bass_kernel_spmd(nc, in_maps, list(range(NCORES)), trace=trace)
    if trace:
        kernel.last_results = res

    acc = np.zeros((NJT, 8, 128, 512), dtype=np.float64)
    for c in range(NCORES):
        acc += res.results[c]["out"].astype(np.float64).reshape(NJT, 8, 128, 512)
    # [st, ec, p, f] -> [st, p, ec, f] -> [S, D]
    out = acc.transpose(0, 2, 1, 3).reshape(S, D).astype(np.float32)
    return out.reshape(B, S, D)


# revision 36
# speedup vs baseline: 1.1279x; 1.0055x over previous
"""GQA attention kernel (B=1, S=2048, D=4096, 32 Q heads / 8 KV heads, RoPE,
causal) for 8 Trainium2 NeuronCores.

Sharding: tensor-parallel over heads. Core c owns Q heads 4c..4c+3 and KV head
c (whole GQA group), computes its context slice and a partial o-projection
(rows 512c..512c+511 of Wo); the host sums the 8 partial outputs.

v4 changes vs v2:
- chunk-1's K/V projection matmuls emitted as PE filler inside chunk-0's
  attention (the previously-idle 35us DVE-bound window); chunk-1's own
  loop then only runs Q0-3 + K/V t16-31
- o-projection filler emitted 2-4 matmuls at a time inside the attention
  tile loop (cursor-based; per-unit paired DMA, 3:1 sync:gpsimd triggers)
- pT / vnat / v-transpose in bf16 (2x DVE rate on ctx normalize, exact
  causal widths on ctx matmuls)
- chunk-0 phase A densified with lag-2 q0/q1 matmuls
- wo/mask/ident/ones loads staged on the scalar queue after chunk-0 evacs
  (clear of the startup DMA crunch, ahead of first o-proj use)
"""
import numpy as np
import ml_dtypes
from contextlib import ExitStack

try:  # reuse compiled executables across processes when possible
    import jax
    jax.config.update("jax_compilation_cache_dir", "/tmp/jax_comp_cache")
    jax.config.update("jax_persistent_cache_min_entry_size_bytes", -1)
    jax.config.update("jax_persistent_cache_min_compile_time_secs", 1.0)
except Exception:
    pass

import concourse.bacc as bacc
import concourse.tile as tile
import concourse.mybir as mybir
from concourse.bass_utils import run_bass_kernel_spmd

F32 = mybir.dt.float32
F32R = mybir.dt.float32r
BF16 = mybir.dt.bfloat16
F16 = mybir.dt.float16

S = 2048            # sequence length
D = 4096            # hidden dim
HD = 128            # head dim
NCORES = 8
QH = 4              # q heads per core
KT = D // 128       # 32 contraction tiles for the projections
NCHUNK = S // 512   # 4 sequence chunks of 512
NJT = S // 128      # 16 seq tiles of 128
INV_SQRT_D = float(1.0 / np.sqrt(np.float32(HD)))
NEG_INF = -3.4e38
ROPE_BASE = 10000.0


def _build_nc():
    nc = bacc.Bacc(None)

    # pre-tiled bf16 inputs: hst rows ((icnk*16 + tp)*128 + p) hold the
    # t-PAIR (2*tp, 2*tp+1) side by side -> one DMA per two tiles
    hst_d = nc.dram_tensor("hst", [NCHUNK * 16 * 128, 1024], BF16, kind="ExternalInput")
    # weights bf16, partition-major [128, KT, m] so group loads are contiguous
    wqb_d = nc.dram_tensor("wqb", [128, KT, QH * HD], BF16, kind="ExternalInput")
    wkb_d = nc.dram_tensor("wkb", [128, KT, HD], BF16, kind="ExternalInput")
    wvb_d = nc.dram_tensor("wvb", [128, KT, HD], BF16, kind="ExternalInput")
    wob_d = nc.dram_tensor("wob", [4 * 128, D], BF16, kind="ExternalInput")
    # cos / sign-folded sin, partition-major per chunk [128, NCHUNK, 512]
    cos_d = nc.dram_tensor("cost", [128, NCHUNK, 512], F16, kind="ExternalInput")
    sinn_d = nc.dram_tensor("sinn", [128, NCHUNK, 512], F16, kind="ExternalInput")
    mask_d = nc.dram_tensor("maskt", [128, 4, 512], BF16, kind="ExternalInput")
    ident_d = nc.dram_tensor("ident", [128, 128], BF16, kind="ExternalInput")
    ones_d = nc.dram_tensor("ones", [128, 128], F32R, kind="ExternalInput")
    onesb_d = nc.dram_tensor("onesb", [128, 128], BF16, kind="ExternalInput")
    # output bf16, tiled rows ((st*8 + ec)*128 + p)
    out_d = nc.dram_tensor("out", [NJT * 8 * 128, 512], BF16, kind="ExternalOutput")

    with tile.TileContext(nc) as tc, ExitStack() as ctx:
        wpool = ctx.enter_context(tc.tile_pool(name="wpool", bufs=1))
        cpool = ctx.enter_context(tc.tile_pool(name="cpool", bufs=1))
        big = ctx.enter_context(tc.tile_pool(name="bigacts", bufs=1))
        trig = ctx.enter_context(tc.tile_pool(name="trig", bufs=2))
        hsp = ctx.enter_context(tc.tile_pool(name="hsp", bufs=12))
        hs0 = ctx.enter_context(tc.tile_pool(name="hs0", bufs=8))
        chp = ctx.enter_context(tc.tile_pool(name="chp", bufs=4))
        rotp = ctx.enter_context(tc.tile_pool(name="rotp", bufs=1))
        qrp = ctx.enter_context(tc.tile_pool(name="qrp", bufs=4))
        tmpp = ctx.enter_context(tc.tile_pool(name="tmpp", bufs=1))
        ptp = ctx.enter_context(tc.tile_pool(name="ptp", bufs=5))
        rsp = ctx.enter_context(tc.tile_pool(name="rsp", bufs=2))
        smal = ctx.enter_context(tc.tile_pool(name="smal", bufs=1))
        rbp = ctx.enter_context(tc.tile_pool(name="rbp", bufs=2))
        osb = ctx.enter_context(tc.tile_pool(name="osb", bufs=4))
        psum = ctx.enter_context(tc.tile_pool(name="psum", bufs=8, space="PSUM"))

        # ---- resident weights & constants (bf16, loaded directly) ----
        wq_sb = wpool.tile([128, KT, QH * HD], BF16, tag="wq")
        wk_sb = wpool.tile([128, KT, HD], BF16, tag="wk")
        wv_sb = wpool.tile([128, KT, HD], BF16, tag="wv")
        wo_sb = wpool.tile([128, 4, D], BF16, tag="wo")

        mask_sb = cpool.tile([128, 4, 512], BF16, tag="mask")
        ident_sb = cpool.tile([128, 128], BF16, tag="ident")
        ones_sb = cpool.tile([128, 128], F32R, tag="ones")
        onesb_sb = cpool.tile([128, 128], BF16, tag="onesb")

        # per-chunk tiles: one [128,512] each per chunk so old-chunk reads
        # never depend on the current chunk's writes (false-dep avoidance)
        krope_cs = [big.tile([128, 512], F32R, tag=f"krope{i}",
                             name=f"krope{i}") for i in range(NCHUNK)]
        vnat_cs = [big.tile([128, 512], BF16, tag=f"vnat{i}",
                            name=f"vnat{i}") for i in range(NCHUNK)]
        ctx_cs = [big.tile([128, QH, 512], BF16, tag=f"ctx{i}",
                           name=f"ctx{i}") for i in range(NCHUNK)]

        # --- weight group loads (scalar queue; chunk-0 only) ---
        def wq_load(g):  # 4 groups of 8 t-tiles, 1 MB each
            nc.scalar.dma_start(out=wq_sb[:, 8 * g:8 * g + 8, :],
                                in_=wqb_d[:, 8 * g:8 * g + 8, :])

        nc.scalar.dma_start(out=wk_sb[:, 0:2, :], in_=wkb_d[:, 0:2, :])
        nc.scalar.dma_start(out=wv_sb[:, 0:2, :], in_=wvb_d[:, 0:2, :])
        nc.scalar.dma_start(out=wk_sb[:, 2:16, :], in_=wkb_d[:, 2:16, :])
        nc.scalar.dma_start(out=wv_sb[:, 2:16, :], in_=wvb_d[:, 2:16, :])

        def _late_loads(phase, tp):
            # chunk-0 only: stream remaining weights
            if phase == 0:
                if tp == 1:
                    wq_load(0)
                elif tp == 2:
                    nc.scalar.dma_start(out=wk_sb[:, 16:32, :],
                                        in_=wkb_d[:, 16:32, :])
                    nc.scalar.dma_start(out=wv_sb[:, 16:32, :],
                                        in_=wvb_d[:, 16:32, :])
                elif tp == 3:
                    wq_load(1)
            else:
                if tp == 0:
                    wq_load(2)
                elif tp == 2:
                    wq_load(3)

        hst_prefetched = {}

        # ---- fine-grained o-projection filler ----
        # each unit (st, ecp) = 2 halves x 4 matmuls; half0 copy on DVE,
        # half1 on scalar, then one paired DMA (3:1 sync:gpsimd triggers).
        # pe_filler(n) emits up to n matmuls, resuming at its cursor.
        out_r = out_d[:, :].rearrange("(n p) f -> p n f", p=128)
        _ost = {"st": 0, "ecp": 0, "half": 0, "jt": 0, "max_st": -1,
                "q": 0, "ot": None, "oacc": None, "done": False}

        def pe_filler(budget):
            n = 0
            while n < budget and not _ost["done"]:
                st, ecp, half, jt = (_ost["st"], _ost["ecp"], _ost["half"],
                                     _ost["jt"])
                if st > _ost["max_st"]:
                    break
                if half == 0 and jt == 0:
                    _ost["ot"] = osb.tile([128, 2, 512], BF16, tag="ot",
                                          name=f"ot{st}_{ecp}")
                if jt == 0:
                    _ost["oacc"] = psum.tile([128, 512], F32, tag="ps",
                                             name=f"o{st}_{ecp}_{half}")
                ec = ecp * 2 + half
                nc.tensor.matmul(
                    _ost["oacc"][:],
                    ctx_cs[st // 4][:, jt, (st % 4) * 128:(st % 4 + 1) * 128],
                    wo_sb[:, jt, ec * 512:(ec + 1) * 512],
                    start=(jt == 0), stop=(jt == 3))
                n += 1
                if jt < 3:
                    _ost["jt"] = jt + 1
                    continue
                if half == 0:
                    nc.vector.tensor_copy(_ost["ot"][:, 0, :], _ost["oacc"][:])
                else:
                    nc.scalar.copy(_ost["ot"][:, 1, :], _ost["oacc"][:])
                    n0 = st * 8 + ecp * 2
                    eng = nc.gpsimd if _ost["q"] % 4 == 3 else nc.sync
                    _ost["q"] += 1
                    eng.dma_start(out=out_r[:, n0:n0 + 2, :], in_=_ost["ot"][:])
                _ost["jt"] = 0
                if half == 0:
                    _ost["half"] = 1
                else:
                    _ost["half"] = 0
                    if ecp < 3:
                        _ost["ecp"] = ecp + 1
                    else:
                        _ost["ecp"] = 0
                        if st + 1 < NJT:
                            _ost["st"] = st + 1
                        else:
                            _ost["done"] = True
            return n

        def hst_pair_load(icnk, tp):
            key = (icnk, tp)
            if key in hst_prefetched:
                return hst_prefetched.pop(key)
            h = hsp.tile([128, 2, 512], BF16, tag="hst", name=f"hst{icnk}_{tp}")
            r0 = (icnk * 16 + tp) * 128
            nc.sync.dma_start(out=h[:], in_=hst_d[r0:r0 + 128, :])
            return h

        # chunk-1 K/V early-start state (filled during chunk-0 attention)
        kv1 = {"accs": None, "pairs": {}, "t": 0}

        def kv1_filler(budget):
            # emit chunk-1 K/V matmuls for prefetched pairs (t = 0..23)
            n = 0
            while n < budget and kv1["t"] < 24:
                t = kv1["t"]
                tp = t // 2
                if tp not in kv1["pairs"]:
                    break
                pair = kv1["pairs"][tp]
                for mi, acc in enumerate(kv1["accs"]):
                    lhsT = wk_sb[:, t, :] if mi == 0 else wv_sb[:, t, :]
                    nc.tensor.matmul(acc[:], lhsT, pair[:, t % 2, :],
                                     start=(t == 0), stop=False)
                    n += 1
                kv1["t"] = t + 1
            return n

        # ---- fused per-chunk pipeline ----
        for icnk in range(NCHUNK):
            # per-chunk trig slices (gpsimd queue, contiguous per partition)
            cos_t = trig.tile([128, 512], F16, tag="cos", name=f"cos{icnk}")
            sinn_t = trig.tile([128, 512], F16, tag="sinn", name=f"sinn{icnk}")
            nc.gpsimd.dma_start(out=cos_t[:], in_=cos_d[:, icnk, :])
            nc.gpsimd.dma_start(out=sinn_t[:], in_=sinn_d[:, icnk, :])

            # joint qkv projection for this chunk: 6 accumulators (q0..q3, k, v)
            if icnk == 1 and kv1["accs"] is not None:
                accs = [psum.tile([128, 512], F32, tag="ps",
                                  name=f"acc{icnk}_{i}") for i in range(4)]
                accs = accs + kv1["accs"]  # K/V groups already open (t 0..15)
            else:
                accs = [psum.tile([128, 512], F32, tag="ps",
                                  name=f"acc{icnk}_{i}") for i in range(6)]
            if icnk == 0:
                # phase A: K/V per arriving pair + lag-2 q0/q1 — the PE
                # starts early and stays denser while weights stream in
                apairs = []
                for tp in range(8):
                    hA = hs0.tile([128, 2, 512], BF16, tag="hs0",
                                  name=f"hsA{tp}")
                    nc.sync.dma_start(out=hA[:], in_=hst_d[tp * 128:
                                                           (tp + 1) * 128, :])
                    apairs.append(hA)
                    _late_loads(0, tp)
                    for tt in range(2):
                        t = 2 * tp + tt
                        nc.tensor.matmul(accs[4][:], wk_sb[:, t, :],
                                         hA[:, tt, :],
                                         start=(t == 0), stop=False)
                        nc.tensor.matmul(accs[5][:], wv_sb[:, t, :],
                                         hA[:, tt, :],
                                         start=(t == 0), stop=False)
                        if tp >= 4:  # q0/q1 join, lagged 2 pairs so a late
                            tl = t - 4  # wq arrival can't block K/V
                            for m in (0, 1):
                                nc.tensor.matmul(
                                    accs[m][:],
                                    wq_sb[:, tl, m * HD:(m + 1) * HD],
                                    apairs[tp - 2][:, tt, :],
                                    start=(tl == 4), stop=False)
                for tp in range(16):
                    if tp < 8:
                        hst_t = apairs[tp]
                    else:
                        hst_t = hst_pair_load(0, tp)
                    _late_loads(1, tp)
                    for tt in range(2):
                        t = 2 * tp + tt
                        morder = ((4, 0, 5, 1, 2, 3) if t == KT - 1
                                  else (0, 1, 2, 3))
                        for m in morder:
                            if m < 4:
                                if m < 2 and 4 <= t < 12:
                                    continue  # done in phase A
                                # q0/q1 opened their psum group in phase A
                                nc.tensor.matmul(
                                    accs[m][:], wq_sb[:, t, m * HD:(m + 1) * HD],
                                    hst_t[:, tt, :],
                                    start=(t == 0 and m >= 2),
                                    stop=(t == KT - 1))
                            elif t >= 16:
                                lhsT = wk_sb[:, t, :] if m == 4 else wv_sb[:, t, :]
                                nc.tensor.matmul(accs[m][:], lhsT,
                                                 hst_t[:, tt, :],
                                                 start=False, stop=(t == KT - 1))
                        if 16 <= t < KT - 1:
                            for m in (4, 5):
                                lhsT = wk_sb[:, t, :] if m == 4 else wv_sb[:, t, :]
                                nc.tensor.matmul(accs[m][:], lhsT,
                                                 hst_t[:, tt, :],
                                                 start=False, stop=False)
            else:
                kv_early = icnk == 1 and kv1["accs"] is not None
                t0kv = kv1["t"] if kv_early else 0  # K/V done up to here
                for tp in range(16):
                    hst_t = hst_pair_load(icnk, tp)
                    for tt in range(2):
                        t = 2 * tp + tt
                        morder = ((4, 0, 5, 1, 2, 3) if t == KT - 1
                                  else (0, 1, 2, 3, 4, 5))
                        for m in morder:
                            if m < 4:
                                lhsT = wq_sb[:, t, m * HD:(m + 1) * HD]
                            elif m == 4:
                                lhsT = wk_sb[:, t, :]
                            else:
                                lhsT = wv_sb[:, t, :]
                            if m >= 4 and t < t0kv:
                                continue  # K/V done in chunk-0 attn filler
                            nc.tensor.matmul(accs[m][:], lhsT, hst_t[:, tt, :],
                                             start=(t == 0 and not
                                                    (m >= 4 and kv_early)),
                                             stop=(t == KT - 1))

            def evac(m, eng="v", dt=F32R):
                ch = chp.tile([128, 512], dt, tag="ch" if dt == F32R else "chb",
                              name=f"ch{icnk}_{m}",
                              bufs=None if dt == F32R else 2)
                if eng == "s":  # scalar engine: parallel to DVE at boundaries
                    nc.scalar.copy(ch[:], accs[m][:])
                else:
                    nc.vector.tensor_copy(ch[:], accs[m][:])
                return ch

            def rope_into(ch, dest_ap, name):
                # rotate_half via partition-shifted copies; sign folded in sinn
                rot = rotp.tile([128, 512], F32, tag="rot", name=f"rot{name}")
                nc.vector.tensor_copy(rot[0:64, :], ch[64:128, :].bitcast(F32))
                nc.vector.tensor_copy(rot[64:128, :], ch[0:64, :].bitcast(F32))
                t1 = tmpp.tile([128, 512], F32, tag="t1", name=f"t1{name}")
                nc.vector.tensor_mul(t1[:], ch[:].bitcast(F32), cos_t[:])
                t2 = tmpp.tile([128, 512], F32, tag="t2", name=f"t2{name}")
                nc.vector.tensor_mul(t2[:], rot[:], sinn_t[:])
                nc.vector.tensor_add(dest_ap, t1[:], t2[:])

            def rope_q(m):
                qr = qrp.tile([128, 512], F32R, tag="qrp", name=f"qr{icnk}_{m}")
                rope_into(chs[m], qr[:], f"q{icnk}_{m}")
                return qr

            # evacuate ALL psum accumulators upfront (frees banks for the
            # attention tiles; lazy evac deadlocks the 8-slot psum rotation),
            # but compose ropes lazily per head so DVE runs just ahead of PE.
            qrs = [None] * QH
            chs = {}
            if icnk == 0:
                chs[4] = evac(4, "s")
                chs[0] = evac(0, "s")
                chs[5] = evac(5, "s", BF16)
                rope_into(chs[4], krope_cs[icnk][:], f"k{icnk}")
                qrs[0] = qrp.tile([128, 512], F32R, tag="qrp",
                                  name=f"qr{icnk}_0")
                rope_into(chs[0], qrs[0][:], f"q{icnk}_0")
                # stage table loads on the scalar queue: they run after
                # chunk-0's evacs, clear of the startup DMA crunch
                nc.scalar.dma_start(out=ident_sb[:], in_=ident_d[:, :])
                nc.scalar.dma_start(out=mask_sb[:], in_=mask_d[:, :, :])
                nc.scalar.dma_start(out=ones_sb[:], in_=ones_d[:, :])
                nc.scalar.dma_start(out=onesb_sb[:], in_=onesb_d[:, :])
            else:
                chs[0] = evac(0, "s")
                chs[4] = evac(4, "s")
                chs[5] = evac(5, "s", BF16)
                qrs[0] = qrp.tile([128, 512], F32R, tag="qrp",
                                  name=f"qr{icnk}_0")
                rope_into(chs[0], qrs[0][:], f"q{icnk}_0")
                rope_into(chs[4], krope_cs[icnk][:], f"k{icnk}")
            ch_v = chs[5]
            vt_ps = psum.tile([128, 512], BF16, tag="ps", name=f"vt{icnk}",
                              padded_shape=[128, 1024])
            for tt in range(4):
                nc.tensor.matmul(vt_ps[:, tt * 128:(tt + 1) * 128],
                                 ch_v[:, tt * 128:(tt + 1) * 128],
                                 ident_sb[:], is_transpose=True,
                                 start=True, stop=True)
            for tt in range(4):
                nc.vector.tensor_copy(vnat_cs[icnk][:, tt * 128:(tt + 1) * 128],
                                      vt_ps[:, tt * 128:(tt + 1) * 128])
            for m in (1, 2, 3):
                chs[m] = evac(m)
            if icnk == 0:
                # open chunk-1 K/V psum groups; their matmuls fill chunk-0's
                # attention stalls as prefetched chunk-1 pairs arrive
                kv1["accs"] = [psum.tile([128, 512], F32, tag="ps",
                                         name=f"acc1_kv{i}") for i in range(2)]
                for tpre in range(12):
                    hh = hsp.tile([128, 2, 512], BF16, tag="hst",
                                  name=f"hst1_{tpre}")
                    r0 = (16 + tpre) * 128
                    nc.sync.dma_start(out=hh[:], in_=hst_d[r0:r0 + 128, :])
                    hst_prefetched[(1, tpre)] = hh
                    kv1["pairs"][tpre] = hh
                kv1_filler(10)  # cover the k/q0 rope window ahead of h0
            if icnk >= 1:
                pe_filler(20)  # cover the k/q0 rope window ahead of h0

            def filler(n):
                if icnk == 0:
                    return kv1_filler(n)
                return pe_filler(n)

            # attention for the 4 heads, query chunk = icnk (keys 0..4icnk+3)
            def attention(h):
                qr = qrs[h]
                ctx_acc = psum.tile([128, 512], F32, tag="ps",
                                    name=f"ctx{icnk}_{h}")
                acc_rs = rsp.tile([128, 512], F32R, tag="rs",
                                  name=f"rs{icnk}_{h}")
                jt_max = icnk * 4 + 3
                pending = []
                LOOKAHEAD = 2

                def consume(item, last):
                    jt, lo, pT = item
                    r = jt - icnk * 4
                    loc = max(lo, 128 * r)  # bf16: exact causal width
                    nc.tensor.matmul(
                        ctx_acc[:, loc:512],
                        vnat_cs[jt // 4][:, (jt % 4) * 128:(jt % 4 + 1) * 128],
                        pT[:, loc:512],
                        start=(jt == 0), stop=last,
                        skip_group_check=True)

                rs_ps = psum.tile([1, 512], F32, tag="ps",
                                  name=f"rsps{icnk}_{h}")
                n_pe_rs = 0
                for jt in range(jt_max + 1):
                    r = jt - icnk * 4
                    lo = 0 if r <= 0 else (128 if r == 1 else 256)
                    sT = psum.tile([128, 512], F32, tag="ps",
                                   name=f"sT{icnk}_{h}_{jt}")
                    nc.tensor.matmul(
                        sT[:, lo:512],
                        krope_cs[jt // 4][:, (jt % 4) * 128:(jt % 4 + 1) * 128],
                        qr[:, lo:512], start=True, stop=True)
                    filler(2)
                    if len(pending) >= LOOKAHEAD:
                        consume(pending.pop(0), False)
                    if r >= 0:  # diagonal: apply causal mask
                        nc.vector.tensor_add(sT[:, lo:512], sT[:, lo:512],
                                             mask_sb[:, r, lo:512])
                    pT = ptp.tile([128, 512], BF16, tag="pt",
                                  name=f"pt{icnk}_{h}_{jt}")
                    nc.scalar.activation(out=pT[:, lo:512], in_=sT[:, lo:512],
                                         func=mybir.ActivationFunctionType.Exp,
                                         scale=INV_SQRT_D)
                    # hybrid rowsum: odd OLD tiles via PE ones-matmul into
                    # rs_ps (sequential groups, one open at a time); the rest
                    # accumulated on DVE (balances engine load in big chunks)
                    if jt == 0:
                        nc.vector.tensor_copy(acc_rs[:, :], pT[:, :])
                    elif r < 0 and jt % 2 == 1:
                        nc.tensor.matmul(rs_ps[:], onesb_sb[:, 0:1], pT[:, :],
                                         start=(n_pe_rs == 0), stop=False,
                                         skip_group_check=True)
                        n_pe_rs += 1
                    else:
                        nc.vector.tensor_add(acc_rs[:, lo:512],
                                             acc_rs[:, lo:512], pT[:, lo:512])
                    pending.append((jt, lo, pT))
                while pending:
                    filler(2)
                    consume(pending.pop(0), len(pending) == 0)

                # final rowsum: add the DVE-accumulated part into rs_ps
                nc.tensor.matmul(rs_ps[:], ones_sb[:, 0:1], acc_rs[:],
                                 start=(n_pe_rs == 0), stop=True,
                                 skip_group_check=True)
                recip = smal.tile([1, 512], F32, tag="recip")
                nc.vector.reciprocal_approx_fast(recip[:], rs_ps[:])
                rb = rbp.tile([128, 512], F32, tag="rb", name=f"rb{icnk}_{h}")
                nc.gpsimd.partition_broadcast(rb[:], recip[:])
                nc.vector.tensor_mul(ctx_cs[icnk][:, h, :], ctx_acc[:], rb[:])

            for h in range(QH):
                if h + 1 < QH:
                    qrs[h + 1] = rope_q(h + 1)  # DVE runs ahead of the PE here
                attention(h)
                filler(4)
                if icnk == 0 and h == 0:
                    # wo streams during the rest of chunk-0's attention
                    for jt in range(4):
                        nc.scalar.dma_start(
                            out=wo_sb[:, jt, :],
                            in_=wob_d[jt * 128:(jt + 1) * 128, :])
                if icnk >= 1 and icnk < NCHUNK - 1 and h == 1:
                    # prefetch next chunk's first hst pairs during attention
                    for tpre in range(6):
                        hh = hsp.tile([128, 2, 512], BF16, tag="hst",
                                      name=f"hst{icnk + 1}_{tpre}")
                        r0 = ((icnk + 1) * 16 + tpre) * 128
                        nc.sync.dma_start(out=hh[:], in_=hst_d[r0:r0 + 128, :])
                        hst_prefetched[(icnk + 1, tpre)] = hh
            # chunk i's ctx complete -> o-proj units for st 4i..4i+3 unlock
            _ost["max_st"] = 4 * icnk + 3

        # ---- drain remaining o-projection units ----
        pe_filler(10 ** 6)

    nc.finalize()
    return nc


_NC_CACHE = None


def _host_tables():
    inv_freq = 1.0 / (ROPE_BASE ** (np.arange(0, HD, 2, dtype=np.float32) / HD))
    pos = np.arange(S, dtype=np.float32)
    freqs = pos[:, None] * inv_freq[None, :].astype(np.float32)   # [S, 64]
    emb = np.concatenate([freqs, freqs], axis=1).astype(np.float32)  # [S, 128]
    cosT = np.ascontiguousarray(np.cos(emb).astype(np.float32).T)  # [128, S]
    sinT = np.ascontiguousarray(np.sin(emb).astype(np.float32).T)
    sinN = sinT.copy()
    sinN[0:64, :] *= -1.0  # sign folded: rot_abs[0:64]=x2, [64:]=x1

    # per-chunk partition-major trig: [128, NCHUNK, 512] fp16
    cosPM = np.ascontiguousarray(cosT.reshape(128, NCHUNK, 512)).astype(np.float16)
    sinnPM = np.ascontiguousarray(sinN.reshape(128, NCHUNK, 512)).astype(np.float16)

    # causal mask for diagonal blocks, transposed orientation [jp, r, if]
    jp = np.arange(128)[:, None, None]
    r = np.arange(4)[None, :, None]
    iF = np.arange(512)[None, None, :]
    mask = np.where(r * 128 + jp <= iF, 0.0, NEG_INF).astype(np.float32)
    mask_bf = mask.astype(ml_dtypes.bfloat16)

    ident = np.eye(128, dtype=np.float32).astype(ml_dtypes.bfloat16)
    ones = np.ones((128, 128), dtype=np.float32)
    onesb = np.ones((128, 128), dtype=np.float32).astype(ml_dtypes.bfloat16)
    return cosPM, sinnPM, mask_bf, ident, ones, onesb


def kernel(hidden_states, Wq, Wk, Wv, Wo):
    global _NC_CACHE
    if _NC_CACHE is None:
        _NC_CACHE = _build_nc()
    nc = _NC_CACHE

    hs = np.asarray(hidden_states, dtype=np.float32)
    B = hs.shape[0]
    assert hs.shape == (B, S, D)
    hst = np.ascontiguousarray(hs[0].T).astype(ml_dtypes.bfloat16)  # [D, S]
    # pair-tiled: row (icnk*16 + tp)*128 + p = [t=2tp | t=2tp+1] halves
    hst_t = np.ascontiguousarray(
        hst.reshape(16, 2, 128, NCHUNK, 512).transpose(3, 0, 2, 1, 4)
    ).reshape(NCHUNK * 16 * 128, 1024)
    cosPM, sinnPM, mask_bf, ident, ones, onesb = _host_tables()

    Wq = np.asarray(Wq, dtype=np.float32)
    Wk = np.asarray(Wk, dtype=np.float32)
    Wv = np.asarray(Wv, dtype=np.float32)
    Wo = np.asarray(Wo, dtype=np.float32)

    def pm_bf16(w):  # [D, m] -> partition-major [128, KT, m] bf16
        m = w.shape[1]
        return np.ascontiguousarray(
            w.reshape(KT, 128, m).transpose(1, 0, 2)).astype(ml_dtypes.bfloat16)

    in_maps = []
    for c in range(NCORES):
        in_maps.append({
            "hst": hst_t,
            "wqb": pm_bf16(Wq[:, c * QH * HD:(c + 1) * QH * HD]),
            "wkb": pm_bf16(Wk[:, c * HD:(c + 1) * HD]),
            "wvb": pm_bf16(Wv[:, c * HD:(c + 1) * HD]),
            "wob": np.ascontiguousarray(
                Wo[c * QH * HD:(c + 1) * QH * HD, :]).astype(ml_dtypes.bfloat16),
            "cost": cosPM,
            "sinn": sinnPM,
            "maskt": mask_bf,
            "ident": ident,
            "ones": ones,
            "onesb": onesb,
        })

    import os
    trace = os.environ.get("KERNEL_TRACE") == "1"
    if trace:
        try:
            import antenv.axon_hooks  # noqa: F401  (profiling hook, optional)
        except ImportError:
            trace = False
    res = run_bass_kernel_spmd(nc, in_maps, list(range(NCORES)), trace=trace)
    if trace:
        kernel.last_results = res

    acc = np.zeros((NJT, 8, 128, 512), dtype=np.float64)
    for c in range(NCORES):
        acc += res.results[c]["out"].astype(np.float64).reshape(NJT, 8, 128, 512)
    # [st, ec, p, f] -> [st, p, ec, f] -> [S, D]
    out = acc.transpose(0, 2, 1, 3).reshape(S, D).astype(np.float32)
    return out.reshape(B, S, D)


# revision 38
# speedup vs baseline: 1.1361x; 1.0073x over previous
"""GQA attention kernel (B=1, S=2048, D=4096, 32 Q heads / 8 KV heads, RoPE,
causal) for 8 Trainium2 NeuronCores.

Sharding: tensor-parallel over heads. Core c owns Q heads 4c..4c+3 and KV head
c (whole GQA group), computes its context slice and a partial o-projection
(rows 512c..512c+511 of Wo); the host sums the 8 partial outputs.

v4 changes vs v2:
- chunk-1's K/V projection matmuls emitted as PE filler inside chunk-0's
  attention (the previously-idle 35us DVE-bound window); chunk-1's own
  loop then only runs Q0-3 + K/V t16-31
- o-projection filler emitted 2-4 matmuls at a time inside the attention
  tile loop (cursor-based; per-unit paired DMA, 3:1 sync:gpsimd triggers)
- pT / vnat / v-transpose in bf16 (2x DVE rate on ctx normalize, exact
  causal widths on ctx matmuls)
- chunk-0 phase A densified with lag-2 q0/q1 matmuls
- wo/mask/ident/ones loads staged on the scalar queue after chunk-0 evacs
  (clear of the startup DMA crunch, ahead of first o-proj use)
"""
import numpy as np
import ml_dtypes
from contextlib import ExitStack

try:  # reuse compiled executables across processes when possible
    import jax
    jax.config.update("jax_compilation_cache_dir", "/tmp/jax_comp_cache")
    jax.config.update("jax_persistent_cache_min_entry_size_bytes", -1)
    jax.config.update("jax_persistent_cache_min_compile_time_secs", 1.0)
except Exception:
    pass

import concourse.bacc as bacc
import concourse.tile as tile
import concourse.mybir as mybir
from concourse.bass_utils import run_bass_kernel_spmd

F32 = mybir.dt.float32
F32R = mybir.dt.float32r
BF16 = mybir.dt.bfloat16
F16 = mybir.dt.float16

S = 2048            # sequence length
D = 4096            # hidden dim
HD = 128            # head dim
NCORES = 8
QH = 4              # q heads per core
KT = D // 128       # 32 contraction tiles for the projections
NCHUNK = S // 512   # 4 sequence chunks of 512
NJT = S // 128      # 16 seq tiles of 128
INV_SQRT_D = float(1.0 / np.sqrt(np.float32(HD)))
NEG_INF = -3.4e38
ROPE_BASE = 10000.0


def _build_nc():
    nc = bacc.Bacc(None)

    # pre-tiled bf16 inputs: hst rows ((icnk*16 + tp)*128 + p) hold the
    # t-PAIR (2*tp, 2*tp+1) side by side -> one DMA per two tiles
    hst_d = nc.dram_tensor("hst", [NCHUNK * 16 * 128, 1024], BF16, kind="ExternalInput")
    # weights bf16, partition-major [128, KT, m] so group loads are contiguous
    wqb_d = nc.dram_tensor("wqb", [128, KT, QH * HD], BF16, kind="ExternalInput")
    wkb_d = nc.dram_tensor("wkb", [128, KT, HD], BF16, kind="ExternalInput")
    wvb_d = nc.dram_tensor("wvb", [128, KT, HD], BF16, kind="ExternalInput")
    wob_d = nc.dram_tensor("wob", [4 * 128, D], BF16, kind="ExternalInput")
    # cos / sign-folded sin, partition-major per chunk [128, NCHUNK, 512]
    cos_d = nc.dram_tensor("cost", [128, NCHUNK, 512], F16, kind="ExternalInput")
    sinn_d = nc.dram_tensor("sinn", [128, NCHUNK, 512], F16, kind="ExternalInput")
    mask_d = nc.dram_tensor("maskt", [128, 4, 512], BF16, kind="ExternalInput")
    ident_d = nc.dram_tensor("ident", [128, 128], BF16, kind="ExternalInput")
    ones_d = nc.dram_tensor("ones", [128, 128], F32R, kind="ExternalInput")
    onesb_d = nc.dram_tensor("onesb", [128, 128], BF16, kind="ExternalInput")
    # output bf16, tiled rows ((st*8 + ec)*128 + p)
    out_d = nc.dram_tensor("out", [NJT * 8 * 128, 512], BF16, kind="ExternalOutput")

    with tile.TileContext(nc) as tc, ExitStack() as ctx:
        wpool = ctx.enter_context(tc.tile_pool(name="wpool", bufs=1))
        cpool = ctx.enter_context(tc.tile_pool(name="cpool", bufs=1))
        big = ctx.enter_context(tc.tile_pool(name="bigacts", bufs=1))
        trig = ctx.enter_context(tc.tile_pool(name="trig", bufs=2))
        hsp = ctx.enter_context(tc.tile_pool(name="hsp", bufs=12))
        hs0 = ctx.enter_context(tc.tile_pool(name="hs0", bufs=8))
        chp = ctx.enter_context(tc.tile_pool(name="chp", bufs=4))
        rotp = ctx.enter_context(tc.tile_pool(name="rotp", bufs=1))
        qrp = ctx.enter_context(tc.tile_pool(name="qrp", bufs=4))
        tmpp = ctx.enter_context(tc.tile_pool(name="tmpp", bufs=1))
        ptp = ctx.enter_context(tc.tile_pool(name="ptp", bufs=5))
        rsp = ctx.enter_context(tc.tile_pool(name="rsp", bufs=2))
        smal = ctx.enter_context(tc.tile_pool(name="smal", bufs=1))
        rbp = ctx.enter_context(tc.tile_pool(name="rbp", bufs=2))
        osb = ctx.enter_context(tc.tile_pool(name="osb", bufs=4))
        psum = ctx.enter_context(tc.tile_pool(name="psum", bufs=8, space="PSUM"))

        # ---- resident weights & constants (bf16, loaded directly) ----
        wq_sb = wpool.tile([128, KT, QH * HD], BF16, tag="wq")
        wk_sb = wpool.tile([128, KT, HD], BF16, tag="wk")
        wv_sb = wpool.tile([128, KT, HD], BF16, tag="wv")
        wo_sb = wpool.tile([128, 4, D], BF16, tag="wo")

        mask_sb = cpool.tile([128, 4, 512], BF16, tag="mask")
        ident_sb = cpool.tile([128, 128], BF16, tag="ident")
        ones_sb = cpool.tile([128, 128], F32R, tag="ones")
        onesb_sb = cpool.tile([128, 128], BF16, tag="onesb")

        # per-chunk tiles: one [128,512] each per chunk so old-chunk reads
        # never depend on the current chunk's writes (false-dep avoidance)
        krope_cs = [big.tile([128, 512], F32R, tag=f"krope{i}",
                             name=f"krope{i}") for i in range(NCHUNK)]
        vnat_cs = [big.tile([128, 512], BF16, tag=f"vnat{i}",
                            name=f"vnat{i}") for i in range(NCHUNK)]
        ctx_cs = [big.tile([128, QH, 512], BF16, tag=f"ctx{i}",
                           name=f"ctx{i}") for i in range(NCHUNK)]

        # --- weight group loads (scalar queue; chunk-0 only) ---
        def wq_load(g):  # 4 groups of 8 t-tiles, 1 MB each
            nc.scalar.dma_start(out=wq_sb[:, 8 * g:8 * g + 8, :],
                                in_=wqb_d[:, 8 * g:8 * g + 8, :])

        nc.scalar.dma_start(out=wk_sb[:, 0:2, :], in_=wkb_d[:, 0:2, :])
        nc.scalar.dma_start(out=wv_sb[:, 0:2, :], in_=wvb_d[:, 0:2, :])
        nc.scalar.dma_start(out=wk_sb[:, 2:16, :], in_=wkb_d[:, 2:16, :])
        nc.scalar.dma_start(out=wv_sb[:, 2:16, :], in_=wvb_d[:, 2:16, :])

        def _late_loads(phase, tp):
            # chunk-0 only: stream remaining weights
            if phase == 0:
                if tp == 1:
                    wq_load(0)
                elif tp == 2:
                    nc.scalar.dma_start(out=wk_sb[:, 16:32, :],
                                        in_=wkb_d[:, 16:32, :])
                    nc.scalar.dma_start(out=wv_sb[:, 16:32, :],
                                        in_=wvb_d[:, 16:32, :])
                elif tp == 3:
                    wq_load(1)
            else:
                if tp == 0:
                    wq_load(2)
                elif tp == 2:
                    wq_load(3)

        hst_prefetched = {}

        # ---- fine-grained o-projection filler ----
        # each unit (st, ecp) = 2 halves x 4 matmuls; half0 copy on DVE,
        # half1 on scalar, then one paired DMA (3:1 sync:gpsimd triggers).
        # pe_filler(n) emits up to n matmuls, resuming at its cursor.
        out_r = out_d[:, :].rearrange("(n p) f -> p n f", p=128)
        _ost = {"st": 0, "ecp": 0, "half": 0, "jt": 0, "max_st": -1,
                "q": 0, "ot": None, "oacc": None, "done": False}

        def pe_filler(budget):
            n = 0
            while n < budget and not _ost["done"]:
                st, ecp, half, jt = (_ost["st"], _ost["ecp"], _ost["half"],
                                     _ost["jt"])
                if st > _ost["max_st"]:
                    break
                if half == 0 and jt == 0:
                    _ost["ot"] = osb.tile([128, 2, 512], BF16, tag="ot",
                                          name=f"ot{st}_{ecp}")
                if jt == 0:
                    _ost["oacc"] = psum.tile([128, 512], F32, tag="ps",
                                             name=f"o{st}_{ecp}_{half}")
                ec = ecp * 2 + half
                nc.tensor.matmul(
                    _ost["oacc"][:],
                    ctx_cs[st // 4][:, jt, (st % 4) * 128:(st % 4 + 1) * 128],
                    wo_sb[:, jt, ec * 512:(ec + 1) * 512],
                    start=(jt == 0), stop=(jt == 3))
                n += 1
                if jt < 3:
                    _ost["jt"] = jt + 1
                    continue
                if half == 0:
                    nc.vector.tensor_copy(_ost["ot"][:, 0, :], _ost["oacc"][:])
                else:
                    nc.scalar.copy(_ost["ot"][:, 1, :], _ost["oacc"][:])
                    n0 = st * 8 + ecp * 2
                    eng = nc.gpsimd if _ost["q"] % 4 == 3 else nc.sync
                    _ost["q"] += 1
                    eng.dma_start(out=out_r[:, n0:n0 + 2, :], in_=_ost["ot"][:])
                _ost["jt"] = 0
                if half == 0:
                    _ost["half"] = 1
                else:
                    _ost["half"] = 0
                    if ecp < 3:
                        _ost["ecp"] = ecp + 1
                    else:
                        _ost["ecp"] = 0
                        if st + 1 < NJT:
                            _ost["st"] = st + 1
                        else:
                            _ost["done"] = True
            return n

        def hst_pair_load(icnk, tp):
            key = (icnk, tp)
            if key in hst_prefetched:
                return hst_prefetched.pop(key)
            h = hsp.tile([128, 2, 512], BF16, tag="hst", name=f"hst{icnk}_{tp}")
            r0 = (icnk * 16 + tp) * 128
            nc.sync.dma_start(out=h[:], in_=hst_d[r0:r0 + 128, :])
            return h

        # chunk-1 K/V early-start state (filled during chunk-0 attention)
        kv1 = {"accs": None, "pairs": {}, "t": 0}

        def kv1_filler(budget):
            # emit chunk-1 K/V matmuls for prefetched pairs (t = 0..23)
            n = 0
            while n < budget and kv1["t"] < 24:
                t = kv1["t"]
                tp = t // 2
                if tp not in kv1["pairs"]:
                    break
                pair = kv1["pairs"][tp]
                for mi, acc in enumerate(kv1["accs"]):
                    lhsT = wk_sb[:, t, :] if mi == 0 else wv_sb[:, t, :]
                    nc.tensor.matmul(acc[:], lhsT, pair[:, t % 2, :],
                                     start=(t == 0), stop=False)
                    n += 1
                kv1["t"] = t + 1
            return n

        # ---- fused per-chunk pipeline ----
        for icnk in range(NCHUNK):
            # per-chunk trig slices (gpsimd queue, contiguous per partition)
            cos_t = trig.tile([128, 512], F16, tag="cos", name=f"cos{icnk}")
            sinn_t = trig.tile([128, 512], F16, tag="sinn", name=f"sinn{icnk}")
            nc.gpsimd.dma_start(out=cos_t[:], in_=cos_d[:, icnk, :])
            nc.gpsimd.dma_start(out=sinn_t[:], in_=sinn_d[:, icnk, :])

            # joint qkv projection for this chunk: 6 accumulators (q0..q3, k, v)
            if icnk == 1 and kv1["accs"] is not None:
                accs = [psum.tile([128, 512], F32, tag="ps",
                                  name=f"acc{icnk}_{i}") for i in range(4)]
                accs = accs + kv1["accs"]  # K/V groups already open (t 0..15)
            else:
                accs = [psum.tile([128, 512], F32, tag="ps",
                                  name=f"acc{icnk}_{i}") for i in range(6)]
            if icnk == 0:
                # phase A: K/V per arriving pair + lag-2 q0/q1 — the PE
                # starts early and stays denser while weights stream in
                apairs = []
                for tp in range(8):
                    hA = hs0.tile([128, 2, 512], BF16, tag="hs0",
                                  name=f"hsA{tp}")
                    nc.sync.dma_start(out=hA[:], in_=hst_d[tp * 128:
                                                           (tp + 1) * 128, :])
                    apairs.append(hA)
                    _late_loads(0, tp)
                    for tt in range(2):
                        t = 2 * tp + tt
                        nc.tensor.matmul(accs[4][:], wk_sb[:, t, :],
                                         hA[:, tt, :],
                                         start=(t == 0), stop=False)
                        nc.tensor.matmul(accs[5][:], wv_sb[:, t, :],
                                         hA[:, tt, :],
                                         start=(t == 0), stop=False)
                        if tp >= 4:  # q0/q1 join, lagged 2 pairs so a late
                            tl = t - 4  # wq arrival can't block K/V
                            for m in (0, 1):
                                nc.tensor.matmul(
                                    accs[m][:],
                                    wq_sb[:, tl, m * HD:(m + 1) * HD],
                                    apairs[tp - 2][:, tt, :],
                                    start=(tl == 4), stop=False)
                for tp in range(16):
                    if tp < 8:
                        hst_t = apairs[tp]
                    else:
                        hst_t = hst_pair_load(0, tp)
                    _late_loads(1, tp)
                    for tt in range(2):
                        t = 2 * tp + tt
                        # t>=16: K/V weights are resident long before wq g2/g3
                        # arrive, so emit them first to ride out the DMA lag
                        morder = ((4, 0, 5, 1, 2, 3) if t == KT - 1
                                  else ((4, 5, 0, 1, 2, 3) if t >= 16
                                        else (0, 1, 2, 3)))
                        for m in morder:
                            if m < 4:
                                if m < 2 and 4 <= t < 12:
                                    continue  # done in phase A
                                # q0/q1 opened their psum group in phase A
                                nc.tensor.matmul(
                                    accs[m][:], wq_sb[:, t, m * HD:(m + 1) * HD],
                                    hst_t[:, tt, :],
                                    start=(t == 0 and m >= 2),
                                    stop=(t == KT - 1))
                            elif t >= 16:
                                lhsT = wk_sb[:, t, :] if m == 4 else wv_sb[:, t, :]
                                nc.tensor.matmul(accs[m][:], lhsT,
                                                 hst_t[:, tt, :],
                                                 start=False, stop=(t == KT - 1))
            else:
                kv_early = icnk == 1 and kv1["accs"] is not None
                t0kv = kv1["t"] if kv_early else 0  # K/V done up to here
                for tp in range(16):
                    hst_t = hst_pair_load(icnk, tp)
                    for tt in range(2):
                        t = 2 * tp + tt
                        morder = ((4, 0, 5, 1, 2, 3) if t == KT - 1
                                  else (0, 1, 2, 3, 4, 5))
                        for m in morder:
                            if m < 4:
                                lhsT = wq_sb[:, t, m * HD:(m + 1) * HD]
                            elif m == 4:
                                lhsT = wk_sb[:, t, :]
                            else:
                                lhsT = wv_sb[:, t, :]
                            if m >= 4 and t < t0kv:
                                continue  # K/V done in chunk-0 attn filler
                            nc.tensor.matmul(accs[m][:], lhsT, hst_t[:, tt, :],
                                             start=(t == 0 and not
                                                    (m >= 4 and kv_early)),
                                             stop=(t == KT - 1))

            def evac(m, eng="v", dt=F32R):
                ch = chp.tile([128, 512], dt, tag="ch" if dt == F32R else "chb",
                              name=f"ch{icnk}_{m}",
                              bufs=None if dt == F32R else 2)
                if eng == "s":  # scalar engine: parallel to DVE at boundaries
                    nc.scalar.copy(ch[:], accs[m][:])
                else:
                    nc.vector.tensor_copy(ch[:], accs[m][:])
                return ch

            def rope_into(ch, dest_ap, name):
                # rotate_half via partition-shifted copies; sign folded in sinn
                rot = rotp.tile([128, 512], F32, tag="rot", name=f"rot{name}")
                nc.vector.tensor_copy(rot[0:64, :], ch[64:128, :].bitcast(F32))
                nc.vector.tensor_copy(rot[64:128, :], ch[0:64, :].bitcast(F32))
                t1 = tmpp.tile([128, 512], F32, tag="t1", name=f"t1{name}")
                nc.vector.tensor_mul(t1[:], ch[:].bitcast(F32), cos_t[:])
                t2 = tmpp.tile([128, 512], F32, tag="t2", name=f"t2{name}")
                nc.vector.tensor_mul(t2[:], rot[:], sinn_t[:])
                nc.vector.tensor_add(dest_ap, t1[:], t2[:])

            def rope_q(m):
                qr = qrp.tile([128, 512], F32R, tag="qrp", name=f"qr{icnk}_{m}")
                rope_into(chs[m], qr[:], f"q{icnk}_{m}")
                return qr

            # evacuate ALL psum accumulators upfront (frees banks for the
            # attention tiles; lazy evac deadlocks the 8-slot psum rotation),
            # but compose ropes lazily per head so DVE runs just ahead of PE.
            qrs = [None] * QH
            chs = {}
            if icnk == 0:
                chs[4] = evac(4, "s")
                chs[0] = evac(0, "s")
                chs[5] = evac(5, "s", BF16)
                rope_into(chs[4], krope_cs[icnk][:], f"k{icnk}")
                qrs[0] = qrp.tile([128, 512], F32R, tag="qrp",
                                  name=f"qr{icnk}_0")
                rope_into(chs[0], qrs[0][:], f"q{icnk}_0")
                # stage table loads on the scalar queue: they run after
                # chunk-0's evacs, clear of the startup DMA crunch
                nc.scalar.dma_start(out=ident_sb[:], in_=ident_d[:, :])
                nc.scalar.dma_start(out=mask_sb[:], in_=mask_d[:, :, :])
                nc.scalar.dma_start(out=ones_sb[:], in_=ones_d[:, :])
                nc.scalar.dma_start(out=onesb_sb[:], in_=onesb_d[:, :])
            else:
                chs[0] = evac(0, "s")
                chs[4] = evac(4, "s")
                chs[5] = evac(5, "s", BF16)
                qrs[0] = qrp.tile([128, 512], F32R, tag="qrp",
                                  name=f"qr{icnk}_0")
                rope_into(chs[0], qrs[0][:], f"q{icnk}_0")
                rope_into(chs[4], krope_cs[icnk][:], f"k{icnk}")
            ch_v = chs[5]
            vt_ps = psum.tile([128, 512], BF16, tag="ps", name=f"vt{icnk}",
                              padded_shape=[128, 1024])
            for tt in range(4):
                nc.tensor.matmul(vt_ps[:, tt * 128:(tt + 1) * 128],
                                 ch_v[:, tt * 128:(tt + 1) * 128],
                                 ident_sb[:], is_transpose=True,
                                 start=True, stop=True)
            for tt in range(4):
                nc.vector.tensor_copy(vnat_cs[icnk][:, tt * 128:(tt + 1) * 128],
                                      vt_ps[:, tt * 128:(tt + 1) * 128])
            for m in (1, 2, 3):
                chs[m] = evac(m)
            if icnk == 0:
                # open chunk-1 K/V psum groups; their matmuls fill chunk-0's
                # attention stalls as prefetched chunk-1 pairs arrive
                kv1["accs"] = [psum.tile([128, 512], F32, tag="ps",
                                         name=f"acc1_kv{i}") for i in range(2)]
                for tpre in range(12):
                    hh = hsp.tile([128, 2, 512], BF16, tag="hst",
                                  name=f"hst1_{tpre}")
                    r0 = (16 + tpre) * 128
                    nc.sync.dma_start(out=hh[:], in_=hst_d[r0:r0 + 128, :])
                    hst_prefetched[(1, tpre)] = hh
                    kv1["pairs"][tpre] = hh
                kv1_filler(10)  # cover the k/q0 rope window ahead of h0
            if icnk >= 1:
                pe_filler(20)  # cover the k/q0 rope window ahead of h0

            def filler(n):
                if icnk == 0:
                    return kv1_filler(n)
                return pe_filler(n)

            # attention for the 4 heads, query chunk = icnk (keys 0..4icnk+3)
            def attention(h):
                qr = qrs[h]
                ctx_acc = psum.tile([128, 512], F32, tag="ps",
                                    name=f"ctx{icnk}_{h}")
                acc_rs = rsp.tile([128, 512], F32R, tag="rs",
                                  name=f"rs{icnk}_{h}")
                jt_max = icnk * 4 + 3
                pending = []
                LOOKAHEAD = 2

                def consume(item, last):
                    jt, lo, pT = item
                    r = jt - icnk * 4
                    loc = max(lo, 128 * r)  # bf16: exact causal width
                    nc.tensor.matmul(
                        ctx_acc[:, loc:512],
                        vnat_cs[jt // 4][:, (jt % 4) * 128:(jt % 4 + 1) * 128],
                        pT[:, loc:512],
                        start=(jt == 0), stop=last,
                        skip_group_check=True)

                rs_ps = psum.tile([1, 512], F32, tag="ps",
                                  name=f"rsps{icnk}_{h}")
                n_pe_rs = 0
                for jt in range(jt_max + 1):
                    r = jt - icnk * 4
                    lo = 0 if r <= 0 else (128 if r == 1 else 256)
                    sT = psum.tile([128, 512], F32, tag="ps",
                                   name=f"sT{icnk}_{h}_{jt}")
                    nc.tensor.matmul(
                        sT[:, lo:512],
                        krope_cs[jt // 4][:, (jt % 4) * 128:(jt % 4 + 1) * 128],
                        qr[:, lo:512], start=True, stop=True)
                    filler(2)
                    if len(pending) >= LOOKAHEAD:
                        consume(pending.pop(0), False)
                    if r >= 0:  # diagonal: apply causal mask
                        nc.vector.tensor_add(sT[:, lo:512], sT[:, lo:512],
                                             mask_sb[:, r, lo:512])
                    pT = ptp.tile([128, 512], BF16, tag="pt",
                                  name=f"pt{icnk}_{h}_{jt}")
                    nc.scalar.activation(out=pT[:, lo:512], in_=sT[:, lo:512],
                                         func=mybir.ActivationFunctionType.Exp,
                                         scale=INV_SQRT_D)
                    # hybrid rowsum: odd OLD tiles via PE ones-matmul into
                    # rs_ps (sequential groups, one open at a time); the rest
                    # accumulated on DVE (balances engine load in big chunks)
                    if jt == 0:
                        nc.vector.tensor_copy(acc_rs[:, :], pT[:, :])
                    elif r < 0 and jt % 2 == 1:
                        nc.tensor.matmul(rs_ps[:], onesb_sb[:, 0:1], pT[:, :],
                                         start=(n_pe_rs == 0), stop=False,
                                         skip_group_check=True)
                        n_pe_rs += 1
                    else:
                        nc.vector.tensor_add(acc_rs[:, lo:512],
                                             acc_rs[:, lo:512], pT[:, lo:512])
                    pending.append((jt, lo, pT))
                while pending:
                    filler(2)
                    consume(pending.pop(0), len(pending) == 0)

                # final rowsum: add the DVE-accumulated part into rs_ps
                nc.tensor.matmul(rs_ps[:], ones_sb[:, 0:1], acc_rs[:],
                                 start=(n_pe_rs == 0), stop=True,
                                 skip_group_check=True)
                recip = smal.tile([1, 512], F32, tag="recip")
                nc.vector.reciprocal_approx_fast(recip[:], rs_ps[:])
                rb = rbp.tile([128, 512], F32, tag="rb", name=f"rb{icnk}_{h}")
                nc.gpsimd.partition_broadcast(rb[:], recip[:])
                nc.vector.tensor_mul(ctx_cs[icnk][:, h, :], ctx_acc[:], rb[:])

            for h in range(QH):
                if h + 1 < QH:
                    qrs[h + 1] = rope_q(h + 1)  # DVE runs ahead of the PE here
                attention(h)
                filler(4)
                if icnk == 0 and h == 0:
                    # wo streams during the rest of chunk-0's attention
                    for jt in range(4):
                        nc.scalar.dma_start(
                            out=wo_sb[:, jt, :],
                            in_=wob_d[jt * 128:(jt + 1) * 128, :])
                if icnk >= 1 and icnk < NCHUNK - 1 and h == 1:
                    # prefetch next chunk's first hst pairs during attention
                    for tpre in range(6):
                        hh = hsp.tile([128, 2, 512], BF16, tag="hst",
                                      name=f"hst{icnk + 1}_{tpre}")
                        r0 = ((icnk + 1) * 16 + tpre) * 128
                        nc.sync.dma_start(out=hh[:], in_=hst_d[r0:r0 + 128, :])
                        hst_prefetched[(icnk + 1, tpre)] = hh
            # chunk i's ctx complete -> o-proj units for st 4i..4i+3 unlock
            _ost["max_st"] = 4 * icnk + 3

        # ---- drain remaining o-projection units ----
        pe_filler(10 ** 6)

    nc.finalize()
    return nc


_NC_CACHE = None


def _host_tables():
    inv_freq = 1.0 / (ROPE_BASE ** (np.arange(0, HD, 2, dtype=np.float32) / HD))
    pos = np.arange(S, dtype=np.float32)
    freqs = pos[:, None] * inv_freq[None, :].astype(np.float32)   # [S, 64]
    emb = np.concatenate([freqs, freqs], axis=1).astype(np.float32)  # [S, 128]
    cosT = np.ascontiguousarray(np.cos(emb).astype(np.float32).T)  # [128, S]
    sinT = np.ascontiguousarray(np.sin(emb).astype(np.float32).T)
    sinN = sinT.copy()
    sinN[0:64, :] *= -1.0  # sign folded: rot_abs[0:64]=x2, [64:]=x1

    # per-chunk partition-major trig: [128, NCHUNK, 512] fp16
    cosPM = np.ascontiguousarray(cosT.reshape(128, NCHUNK, 512)).astype(np.float16)
    sinnPM = np.ascontiguousarray(sinN.reshape(128, NCHUNK, 512)).astype(np.float16)

    # causal mask for diagonal blocks, transposed orientation [jp, r, if]
    jp = np.arange(128)[:, None, None]
    r = np.arange(4)[None, :, None]
    iF = np.arange(512)[None, None, :]
    mask = np.where(r * 128 + jp <= iF, 0.0, NEG_INF).astype(np.float32)
    mask_bf = mask.astype(ml_dtypes.bfloat16)

    ident = np.eye(128, dtype=np.float32).astype(ml_dtypes.bfloat16)
    ones = np.ones((128, 128), dtype=np.float32)
    onesb = np.ones((128, 128), dtype=np.float32).astype(ml_dtypes.bfloat16)
    return cosPM, sinnPM, mask_bf, ident, ones, onesb


def kernel(hidden_states, Wq, Wk, Wv, Wo):
    global _NC_CACHE
    if _NC_CACHE is None:
        _NC_CACHE = _build_nc()
    nc = _NC_CACHE

    hs = np.asarray(hidden_states, dtype=np.float32)
    B = hs.shape[0]
    assert hs.shape == (B, S, D)
    hst = np.ascontiguousarray(hs[0].T).astype(ml_dtypes.bfloat16)  # [D, S]
    # pair-tiled: row (icnk*16 + tp)*128 + p = [t=2tp | t=2tp+1] halves
    hst_t = np.ascontiguousarray(
        hst.reshape(16, 2, 128, NCHUNK, 512).transpose(3, 0, 2, 1, 4)
    ).reshape(NCHUNK * 16 * 128, 1024)
    cosPM, sinnPM, mask_bf, ident, ones, onesb = _host_tables()

    Wq = np.asarray(Wq, dtype=np.float32)
    Wk = np.asarray(Wk, dtype=np.float32)
    Wv = np.asarray(Wv, dtype=np.float32)
    Wo = np.asarray(Wo, dtype=np.float32)

    def pm_bf16(w):  # [D, m] -> partition-major [128, KT, m] bf16
        m = w.shape[1]
        return np.ascontiguousarray(
            w.reshape(KT, 128, m).transpose(1, 0, 2)).astype(ml_dtypes.bfloat16)

    in_maps = []
    for c in range(NCORES):
        in_maps.append({
            "hst": hst_t,
            "wqb": pm_bf16(Wq[:, c * QH * HD:(c + 1) * QH * HD]),
            "wkb": pm_bf16(Wk[:, c * HD:(c + 1) * HD]),
            "wvb": pm_bf16(Wv[:, c * HD:(c + 1) * HD]),
            "wob": np.ascontiguousarray(
                Wo[c * QH * HD:(c + 1) * QH * HD, :]).astype(ml_dtypes.bfloat16),
            "cost": cosPM,
            "sinn": sinnPM,
            "maskt": mask_bf,
            "ident": ident,
            "ones": ones,
            "onesb": onesb,
        })

    import os
    trace = os.environ.get("KERNEL_TRACE") == "1"
    if trace:
        try:
            import antenv.axon_hooks  # noqa: F401  (profiling hook, optional)
        except ImportError:
            trace = False
    res = run_bass_kernel_spmd(nc, in_maps, list(range(NCORES)), trace=trace)
    if trace:
        kernel.last_results = res

    acc = np.zeros((NJT, 8, 128, 512), dtype=np.float64)
    for c in range(NCORES):
        acc += res.results[c]["out"].astype(np.float64).reshape(NJT, 8, 128, 512)
    # [st, ec, p, f] -> [st, p, ec, f] -> [S, D]
    out = acc.transpose(0, 2, 1, 3).reshape(S, D).astype(np.float32)
    return out.reshape(B, S, D)
